# revision 28
# baseline (speedup 1.0000x reference)
"""Trainium2 Bass kernel for nn_MultiHeadAttention_65987877535893.

MHA with RoPE: B=2, S=2048, D=1024, H=16, Dh=64, causal mask.

Sharding (8 cores): data-parallel over B (x2) x tensor-parallel over heads
(x4 -> 4 heads/core).  Each core computes, for its batch b and head group g:
  QKV projections (column-sharded W), RoPE, causal attention, and a partial
  output  A_g @ Wo_g  (row-sharded Wo).  Host sums the 4 partials per batch.

v2 design (all matmul inputs bf16, fp32 PSUM accumulation):
  - projections: lhsT=W chunks (bf16, FWL), rhs = xT slices; Q/K into a
    combined [128,2,512] PSUM tile (x1 dims half 0, x2 dims half 1).
  - RoPE on DVE: 2 fused muls against [cc|ss] / [ss|cc] tables, then 8
    strided sub/add producing head-contiguous bf16 qr/kr tiles.
  - scores TRANSPOSED (ST[k,q] = K @ Q^T), two heads concurrently via PE
    row-tiling (K=64 each, base partitions 0/64); causal mask handled by
    (a) skipping out-of-range blocks, (b) accumulating a constant -3200
    triangular tile into the 128-wide diagonal boundary via an extra
    identity-lhsT matmul (so exp gives exact zeros) -- no DVE mask pass.
  - exp on ScalarE (scale=1/8 folded), bf16 probs.
  - PV: lhsT=[V_h | ones] [128,65] bf16 -> A^T[64,q] + softmax denominator.
  - normalize: reciprocal on DVE, partition-broadcast on GpSimd (attn
    ucode library), two DVE muls -> bf16 atn.
  - output: O = atn^T @ Wo in PSUM, copied to SBUF bf16 (DVE+ACT split),
    DMA'd out as bf16 partials; host sums in fp32.
  PSUM budget: st(2 tags x2 bufs=4 banks) + at(2) + pq(2) = 8 banks.
"""

import os
import sys

sys.path.insert(0, "/opt/trn_rl_repo")
os.environ.setdefault("MYCRO_LOCAL_CACHE", "1")

import numpy as np
import ml_dtypes

import concourse.bass as bass
import concourse.bacc as bacc
import concourse.mybir as mybir
import concourse.tile as tile
from concourse import library_config
from concourse.bass_utils import run_bass_kernel_spmd

F32 = mybir.dt.float32
BF16 = mybir.dt.bfloat16

B, S, D = 2, 2048, 1024
H = 16
DH = 64
HPC = 4           # heads per core
DG = HPC * DH     # 256
N_CORES = 8
KO = D // 128     # 8 contraction chunks
N_SLICES = S // 512   # 4 q/s slices
EXP_SCALE = float(DH) ** -0.5  # 0.125
MASK_VAL = -3200.0
Exp = mybir.ActivationFunctionType.Exp

USE_GPSIMD_BCAST = True


def build_nc():
    nc = bacc.Bacc()

    xT = nc.dram_tensor("xT", [D, S], BF16, kind="ExternalInput")
    wq1 = nc.dram_tensor("wq1", [D, 128], BF16, kind="ExternalInput")
    wq2 = nc.dram_tensor("wq2", [D, 128], BF16, kind="ExternalInput")
    wk1 = nc.dram_tensor("wk1", [D, 128], BF16, kind="ExternalInput")
    wk2 = nc.dram_tensor("wk2", [D, 128], BF16, kind="ExternalInput")
    wv = nc.dram_tensor("wv", [D, DG], BF16, kind="ExternalInput")
    wo = nc.dram_tensor("wo", [DG, D], BF16, kind="ExternalInput")
    ccss = nc.dram_tensor("ccss", [128, 2, S], BF16, kind="ExternalInput")
    sscc = nc.dram_tensor("sscc", [128, 2, S], BF16, kind="ExternalInput")
    ident = nc.dram_tensor("ident", [128, 128], BF16, kind="ExternalInput")
    maskc = nc.dram_tensor("maskc", [128, 128], BF16, kind="ExternalInput")
    o_part = nc.dram_tensor("o_part", [S, D], BF16, kind="ExternalOutput")

    with tile.TileContext(nc) as tc:
        import contextlib

        ctx = contextlib.ExitStack()
        with ctx:
            if USE_GPSIMD_BCAST:
                nc.gpsimd.load_library(library_config.attn)

            persist = ctx.enter_context(tc.tile_pool(name="persist", bufs=1))
            work = ctx.enter_context(tc.tile_pool(name="work", bufs=2))

            # ---- persistent SBUF tensors ----
            qr = [[persist.tile([128, 512], BF16, tag=f"qr{p}_{ms}", name=f"qr{p}_{ms}")
                   for ms in range(N_SLICES)] for p in range(2)]
            kr = [[persist.tile([128, 512], BF16, tag=f"kr{p}_{ms}", name=f"kr{p}_{ms}")
                   for ms in range(N_SLICES)] for p in range(2)]
            # V with a ones column per (kb, head): [128, kb_in_slice, head, 65]
            vt = [persist.tile([128, 4, HPC, DH + 1], BF16, tag=f"vt{ms}", name=f"vt{ms}")
                  for ms in range(N_SLICES)]
            atn = [[persist.tile([128, 512], BF16, tag=f"atn{p}_{ms}", name=f"atn{p}_{ms}")
                    for ms in range(N_SLICES)] for p in range(2)]
            ccss_sb = persist.tile([128, 2, S], BF16, tag="ccss", name="ccss_sb")
            sscc_sb = persist.tile([128, 2, S], BF16, tag="sscc", name="sscc_sb")
            id_sb = persist.tile([128, 128], BF16, tag="id", name="id_sb")
            mc_sb = persist.tile([128, 128], BF16, tag="mc", name="mc_sb")
            wq1_sb = persist.tile([128, KO, 128], BF16, tag="wq1", name="wq1_sb")
            wq2_sb = persist.tile([128, KO, 128], BF16, tag="wq2", name="wq2_sb")
            wk1_sb = persist.tile([128, KO, 128], BF16, tag="wk1", name="wk1_sb")
            wk2_sb = persist.tile([128, KO, 128], BF16, tag="wk2", name="wk2_sb")
            wv_sb = persist.tile([128, KO, DG], BF16, tag="wv", name="wv_sb")
            wo_sb = persist.tile([128, 2, D], BF16, tag="wo", name="wo_sb")

            # preload order tuned for the m=0 critical path: Q weights,
            # slice-0 rope tables, K weights, V weights, then the rest
            wq1r = wq1.rearrange("(ko p) m -> p ko m", p=128)
            nc.scalar.dma_start(wq1_sb[:, 0:2], wq1r[:, 0:2])
            nc.scalar.dma_start(wq1_sb[:, 2:8], wq1r[:, 2:8])
            nc.scalar.dma_start(
                wq2_sb[:], wq2.rearrange("(ko p) m -> p ko m", p=128))
            nc.scalar.dma_start(ccss_sb[:, :, 0:512], ccss[:, :, 0:512])
            nc.gpsimd.dma_start(sscc_sb[:, :, 0:512], sscc[:, :, 0:512])
            nc.gpsimd.dma_start(
                wk1_sb[:], wk1.rearrange("(ko p) m -> p ko m", p=128))
            nc.gpsimd.dma_start(
                wk2_sb[:], wk2.rearrange("(ko p) m -> p ko m", p=128))
            nc.scalar.dma_start(
                wv_sb[:], wv.rearrange("(ko p) m -> p ko m", p=128))
            nc.sync.dma_start(id_sb[:], ident[:])
            nc.sync.dma_start(mc_sb[:], maskc[:])
            for ms in range(1, N_SLICES):
                sl = slice(512 * ms, 512 * (ms + 1))
                nc.scalar.dma_start(ccss_sb[:, :, sl], ccss[:, :, sl])
                nc.gpsimd.dma_start(sscc_sb[:, :, sl], sscc[:, :, sl])
            nc.scalar.dma_start(
                wo_sb[:], wo.rearrange("(ko p) m -> p ko m", p=128))

            # ones column of vt
            for ms in range(N_SLICES):
                nc.vector.memset(vt[ms][:, :, :, DH], 1.0)

            if True:
                F32R = mybir.dt.float32r
                onesf = persist.tile([128, 64], F32, tag="onesf", name="onesf")
                ones1 = persist.tile([1, 64], F32R, tag="ones1", name="ones1")
                nc.vector.memset(onesf[:], 1.0)
                nc.vector.tensor_copy(ones1[:], onesf[0:1, :])

            # ---- PSUM pools ----
            pqps = ctx.enter_context(
                tc.tile_pool(name="pqps", bufs=1, space="PSUM"))
            stps = ctx.enter_context(
                tc.tile_pool(name="stps", bufs=2, space="PSUM"))
            atps = ctx.enter_context(
                tc.tile_pool(name="atps", bufs=1, space="PSUM"))

            def rope_proj(m, xts, w1_sb, w2_sb, dst):
                sl = slice(512 * m, 512 * (m + 1))
                if True:
                    pq = pqps.tile([128, 2, 512], F32, tag="pq", name="pq")
                    for half, w_sb in ((0, w1_sb), (1, w2_sb)):
                        for ko in range(KO):
                            nc.tensor.matmul(pq[:, half, :], w_sb[:, ko],
                                             xts[:, ko],
                                             start=(ko == 0), stop=(ko == KO - 1))
                    # rope: t12 = [x1*cos | x2*sin], t43 = [x1*sin | x2*cos]
                    t12 = work.tile([128, 2, 512], BF16, tag="t12", name="t12")
                    t43 = work.tile([128, 2, 512], BF16, tag="t43", name="t43")
                    nc.vector.tensor_mul(t12[:], pq[:], ccss_sb[:, :, sl])
                    nc.vector.tensor_mul(t43[:], pq[:], sscc_sb[:, :, sl])
                    for h in range(HPC):
                        pr, a = h // 2, h % 2
                        hs = slice(32 * h, 32 * h + 32)
                        nc.vector.tensor_sub(
                            dst[pr][m][64 * a:64 * a + 32, :],
                            t12[hs, 0], t12[hs, 1])
                        nc.vector.tensor_add(
                            dst[pr][m][64 * a + 32:64 * a + 64, :],
                            t43[hs, 1], t43[hs, 0])

            def proj_q(m):
                """x slice DMA + Q projection + RoPE for slice m."""
                xts = work.tile([128, KO, 512], BF16, tag="xts", name="xts")
                xTr = xT.rearrange("(ko p) s -> p ko s", p=128)
                sl = slice(512 * m, 512 * (m + 1))
                nc.sync.dma_start(xts[:, 0:2], xTr[:, 0:2, sl])
                nc.sync.dma_start(xts[:, 2:5], xTr[:, 2:5, sl])
                nc.sync.dma_start(xts[:, 5:8], xTr[:, 5:8, sl])
                rope_proj(m, xts, wq1_sb, wq2_sb, qr)
                return xts

            def proj_kv(m, xts):
                """K projection + RoPE, V projection for slice m."""
                rope_proj(m, xts, wk1_sb, wk2_sb, kr)
                for scp in range(2):
                    pv = pqps.tile([128, 2, 512], F32, tag="pq", name="pv")
                    for sc2 in range(2):
                        sc = 2 * scp + sc2
                        for ko in range(KO):
                            nc.tensor.matmul(
                                pv[:, sc2, 0:DG],
                                xts[:, ko, 128 * sc:128 * sc + 128],
                                wv_sb[:, ko],
                                start=(ko == 0), stop=(ko == KO - 1))
                    nc.vector.tensor_copy(
                        vt[m][:, 2 * scp:2 * scp + 2, :, 0:DH],
                        pv[:, :, 0:DG].rearrange("p s (h d) -> p s h d", d=DH))

            def score_unit(m, p, kb):
                """Emit scores + mask inject + exp for (m, p, kb); return the
                bf16 probs tile and the slice written."""
                km, j = kb // 4, kb % 4
                ksl = slice(128 * j, 128 * j + 128)
                diag = (km == m)
                c0 = 128 * j if diag else 0
                st = stps.tile([128, 2, 512], F32, tag="st", name="st")
                for a in range(2):
                    nc.tensor.matmul(
                        st[:, a, c0:],
                        kr[p][km][64 * a:64 * a + 64, ksl],
                        qr[p][m][64 * a:64 * a + 64, c0:],
                        start=True, stop=not diag)
                if diag:
                    for a in range(2):
                        nc.tensor.matmul(
                            st[:, a, c0:c0 + 128], id_sb[:], mc_sb[:],
                            start=False, stop=True,
                            skip_group_check=True)
                pt = work.tile([128, 2, 512], BF16, tag="pt",
                               name="pt", bufs=6)
                nc.scalar.activation(
                    pt[:, :, c0:], st[:, :, c0:], Exp, scale=EXP_SCALE)
                return pt, c0

            def pv_unit(m, p, kb, n_kb, at, pt, c0):
                km, j = kb // 4, kb % 4
                for a in range(2):
                    nc.tensor.matmul(
                        at[:, a, c0:], vt[km][:, j, 2 * p + a],
                        pt[:, a, c0:],
                        start=(kb == 0), stop=(kb == n_kb - 1))

            def attention(m):
                # software-pipelined across both p-chains: scores/exp run one
                # unit ahead of PV so the PE never head-of-line blocks on exp
                n_kb = 4 * m + 4
                units = [(p, kb) for p in range(2) for kb in range(n_kb)]
                ats = {}
                pending = []
                normalized = []

                def do_normalize(p, at):
                    normalize(m, p, at, last=(m == 3 and p == 1))
                    normalized.append(p)

                for i, (p, kb) in enumerate(units):
                    if kb == 0:
                        ats[p] = atps.tile([DH + 1, 2, 512], F32, tag="at",
                                           name="at")
                    pt, c0 = score_unit(m, p, kb)
                    pending.append((p, kb, pt, c0))
                    if len(pending) > 3:
                        pp, pkb, ppt, pc0 = pending.pop(0)
                        pv_unit(m, pp, pkb, n_kb, ats[pp], ppt, pc0)
                        if pkb == n_kb - 1:
                            do_normalize(pp, ats[pp])
                while pending:
                    pp, pkb, ppt, pc0 = pending.pop(0)
                    pv_unit(m, pp, pkb, n_kb, ats[pp], ppt, pc0)
                    if pkb == n_kb - 1:
                        do_normalize(pp, ats[pp])
            def normalize(m, p, at, last=False):
                # atn = at[0:64] * recip(denominator row)
                if USE_GPSIMD_BCAST and not last:
                    dsum = work.tile([1, 2, 512], F32, tag="dsum", name="dsum")
                    nc.vector.tensor_copy(dsum[:], at[DH:DH + 1])
                    dbc = work.tile([64, 2, 512], F32, tag="dbc", name="dbc")
                    nc.gpsimd.partition_broadcast(dbc[:], dsum[:])
                    rbc = work.tile([64, 2, 512], F32, tag="rbc", name="rbc")
                    nc.vector.reciprocal_approx_fast(rbc[:], dbc[:])
                    for a in range(2):
                        nc.vector.tensor_mul(
                            atn[p][m][64 * a:64 * a + 64, :],
                            at[0:DH, a], rbc[:, a])
                else:
                    F32R = mybir.dt.float32r
                    for a in range(2):
                        ssum = work.tile([1, 512], F32R, tag="ssum",
                                         name="ssum")
                        nc.vector.tensor_copy(ssum[:], at[DH:DH + 1, a])
                        sbc = stps.tile([64, 512], F32, tag="st",
                                        name="sbc")
                        nc.tensor.matmul(sbc[:], ones1[:], ssum[:],
                                         start=True, stop=True)
                        rbc = work.tile([64, 512], F32, tag="rbc",
                                        name="rbc")
                        nc.vector.reciprocal_approx_fast(rbc[:], sbc[:])
                        nc.vector.tensor_mul(
                            atn[p][m][64 * a:64 * a + 64, :],
                            at[0:DH, a], rbc[:])

            def outproj(m):
                for sc in range(4 * m, 4 * m + 4):
                    scl = slice(128 * (sc % 4), 128 * (sc % 4) + 128)
                    if m == 3 and sc % 2 == 1:
                        # tail: attention is drained, st slots are free --
                        # ping-pong po between pools so osb copies overlap MMs
                        po = stps.tile([128, 2, 512], F32, tag="st", name="po")
                    else:
                        po = pqps.tile([128, 2, 512], F32, tag="pq", name="po")
                    for ks in range(2):
                        for nh in range(2):
                            nc.tensor.matmul(
                                po[:, nh, :], atn[ks][sc // 4][:, scl],
                                wo_sb[:, ks, 512 * nh:512 * nh + 512],
                                start=(ks == 0), stop=(ks == 1))
                    osb = work.tile([128, 1024], BF16, tag="osb", name="osb")
                    nc.scalar.copy(
                        osb[:].rearrange("p (x n) -> p x n", x=2), po[:])
                    nc.sync.dma_start(o_part[128 * sc:128 * sc + 128, :], osb[:])

            xts0 = proj_q(0)
            proj_kv(0, xts0)
            xts_next = proj_q(1)
            attention(0)
            proj_kv(1, xts_next)
            outproj(0)
            xts_next = proj_q(2)
            attention(1)
            proj_kv(2, xts_next)
            outproj(1)
            xts_next = proj_q(3)
            attention(2)
            proj_kv(3, xts_next)
            outproj(2)
            attention(3)
            outproj(3)

    nc.finalize()
    return nc


def prep_inputs(hidden_states, cos, sin, attention_mask, Wq, Wk, Wv, Wo):
    """Host-side sharding/layout prep. Returns in_maps for the 8 cores."""
    bf = ml_dtypes.bfloat16
    hs = np.asarray(hidden_states, dtype=np.float32)
    cos = np.asarray(cos, dtype=np.float32)
    sin = np.asarray(sin, dtype=np.float32)
    Wq = np.asarray(Wq, dtype=np.float32)
    Wk = np.asarray(Wk, dtype=np.float32)
    Wv = np.asarray(Wv, dtype=np.float32)
    Wo = np.asarray(Wo, dtype=np.float32)

    # ccss[p, 0, s] = cos[s, p%32]; ccss[p, 1, s] = sin[s, p%32]
    ct = np.tile(cos.T, (4, 1))  # [128, S]
    st_ = np.tile(sin.T, (4, 1))
    ccss = np.ascontiguousarray(np.stack([ct, st_], axis=1).astype(bf))
    sscc = np.ascontiguousarray(np.stack([st_, ct], axis=1).astype(bf))

    idm = np.eye(128, dtype=bf)
    kappa = np.arange(128)[:, None]
    u = np.arange(128)[None, :]
    maskc = np.where(u >= kappa, 0.0, MASK_VAL).astype(bf)

    xTs = [np.ascontiguousarray(hs[b].T.astype(bf)) for b in range(B)]

    in_maps = []
    for c in range(N_CORES):
        b, g = c // 4, c % 4
        hsl = slice(DG * g, DG * (g + 1))
        wq_g = Wq[:, hsl].reshape(D, HPC, DH)
        wk_g = Wk[:, hsl].reshape(D, HPC, DH)
        in_maps.append({
            "xT": xTs[b],
            "wq1": np.ascontiguousarray(
                wq_g[:, :, :32].reshape(D, 128).astype(bf)),
            "wq2": np.ascontiguousarray(
                wq_g[:, :, 32:].reshape(D, 128).astype(bf)),
            "wk1": np.ascontiguousarray(
                wk_g[:, :, :32].reshape(D, 128).astype(bf)),
            "wk2": np.ascontiguousarray(
                wk_g[:, :, 32:].reshape(D, 128).astype(bf)),
            "wv": np.ascontiguousarray(Wv[:, hsl].astype(bf)),
            "wo": np.ascontiguousarray(Wo[hsl, :].astype(bf)),
            "ccss": ccss,
            "sscc": sscc,
            "ident": idm,
            "maskc": maskc,
        })
    return in_maps


_NC_CACHE = {}


def get_nc():
    if "nc" not in _NC_CACHE:
        _NC_CACHE["nc"] = build_nc()
    return _NC_CACHE["nc"]


def run(inputs, trace=False):
    """Returns (output [B,S,D] fp32, BassKernelResults)."""
    nc = get_nc()
    in_maps = prep_inputs(**inputs)
    res = run_bass_kernel_spmd(nc, in_maps, list(range(N_CORES)), trace=trace)
    out = np.zeros((B, S, D), dtype=np.float32)
    for c in range(N_CORES):
        out[c // 4] += res.results[c]["o_part"].astype(np.float32)
    return out, res


def kernel(**inputs):
    return run(inputs, trace=False)[0]


# revision 29
# speedup vs baseline: 1.0004x; 1.0004x over previous
"""Trainium2 Bass kernel for nn_MultiHeadAttention_65987877535893.

MHA with RoPE: B=2, S=2048, D=1024, H=16, Dh=64, causal mask.

Sharding (8 cores): data-parallel over B (x2) x tensor-parallel over heads
(x4 -> 4 heads/core).  Each core computes, for its batch b and head group g:
  QKV projections (column-sharded W), RoPE, causal attention, and a partial
  output  A_g @ Wo_g  (row-sharded Wo).  Host sums the 4 partials per batch.

v2 design (all matmul inputs bf16, fp32 PSUM accumulation):
  - projections: lhsT=W chunks (bf16, FWL), rhs = xT slices; Q/K into a
    combined [128,2,512] PSUM tile (x1 dims half 0, x2 dims half 1).
  - RoPE on DVE: 2 fused muls against [cc|ss] / [ss|cc] tables, then 8
    strided sub/add producing head-contiguous bf16 qr/kr tiles.
  - scores TRANSPOSED (ST[k,q] = K @ Q^T), two heads concurrently via PE
    row-tiling (K=64 each, base partitions 0/64); causal mask handled by
    (a) skipping out-of-range blocks, (b) accumulating a constant -3200
    triangular tile into the 128-wide diagonal boundary via an extra
    identity-lhsT matmul (so exp gives exact zeros) -- no DVE mask pass.
  - exp on ScalarE (scale=1/8 folded), bf16 probs.
  - PV: lhsT=[V_h | ones] [128,65] bf16 -> A^T[64,q] + softmax denominator.
  - normalize: reciprocal on DVE, partition-broadcast on GpSimd (attn
    ucode library), two DVE muls -> bf16 atn.
  - output: O = atn^T @ Wo in PSUM, copied to SBUF bf16 (DVE+ACT split),
    DMA'd out as bf16 partials; host sums in fp32.
  PSUM budget: st(2 tags x2 bufs=4 banks) + at(2) + pq(2) = 8 banks.
"""

import os
import sys

sys.path.insert(0, "/opt/trn_rl_repo")
os.environ.setdefault("MYCRO_LOCAL_CACHE", "1")

import numpy as np
import ml_dtypes

import concourse.bass as bass
import concourse.bacc as bacc
import concourse.mybir as mybir
import concourse.tile as tile
from concourse import library_config
from concourse.bass_utils import run_bass_kernel_spmd

F32 = mybir.dt.float32
BF16 = mybir.dt.bfloat16

B, S, D = 2, 2048, 1024
H = 16
DH = 64
HPC = 4           # heads per core
DG = HPC * DH     # 256
N_CORES = 8
KO = D // 128     # 8 contraction chunks
N_SLICES = S // 512   # 4 q/s slices
EXP_SCALE = float(DH) ** -0.5  # 0.125
MASK_VAL = -3200.0
Exp = mybir.ActivationFunctionType.Exp

USE_GPSIMD_BCAST = True


def build_nc():
    nc = bacc.Bacc()

    xT = nc.dram_tensor("xT", [D, S], BF16, kind="ExternalInput")
    wq1 = nc.dram_tensor("wq1", [D, 128], BF16, kind="ExternalInput")
    wq2 = nc.dram_tensor("wq2", [D, 128], BF16, kind="ExternalInput")
    wk1 = nc.dram_tensor("wk1", [D, 128], BF16, kind="ExternalInput")
    wk2 = nc.dram_tensor("wk2", [D, 128], BF16, kind="ExternalInput")
    wv = nc.dram_tensor("wv", [D, DG], BF16, kind="ExternalInput")
    wo = nc.dram_tensor("wo", [DG, D], BF16, kind="ExternalInput")
    ccss = nc.dram_tensor("ccss", [128, 2, S], BF16, kind="ExternalInput")
    sscc = nc.dram_tensor("sscc", [128, 2, S], BF16, kind="ExternalInput")
    ident = nc.dram_tensor("ident", [128, 128], BF16, kind="ExternalInput")
    maskc = nc.dram_tensor("maskc", [128, 128], BF16, kind="ExternalInput")
    o_part = nc.dram_tensor("o_part", [S, D], BF16, kind="ExternalOutput")

    with tile.TileContext(nc) as tc:
        import contextlib

        ctx = contextlib.ExitStack()
        with ctx:
            if USE_GPSIMD_BCAST:
                nc.gpsimd.load_library(library_config.attn)

            persist = ctx.enter_context(tc.tile_pool(name="persist", bufs=1))
            work = ctx.enter_context(tc.tile_pool(name="work", bufs=2))

            # ---- persistent SBUF tensors ----
            qr = [[persist.tile([128, 512], BF16, tag=f"qr{p}_{ms}", name=f"qr{p}_{ms}")
                   for ms in range(N_SLICES)] for p in range(2)]
            kr = [[persist.tile([128, 512], BF16, tag=f"kr{p}_{ms}", name=f"kr{p}_{ms}")
                   for ms in range(N_SLICES)] for p in range(2)]
            # V with a ones column per (kb, head): [128, kb_in_slice, head, 65]
            vt = [persist.tile([128, 4, HPC, DH + 1], BF16, tag=f"vt{ms}", name=f"vt{ms}")
                  for ms in range(N_SLICES)]
            atn = [[persist.tile([128, 512], BF16, tag=f"atn{p}_{ms}", name=f"atn{p}_{ms}")
                    for ms in range(N_SLICES)] for p in range(2)]
            ccss_sb = persist.tile([128, 2, S], BF16, tag="ccss", name="ccss_sb")
            sscc_sb = persist.tile([128, 2, S], BF16, tag="sscc", name="sscc_sb")
            id_sb = persist.tile([128, 128], BF16, tag="id", name="id_sb")
            mc_sb = persist.tile([128, 128], BF16, tag="mc", name="mc_sb")
            wq1_sb = persist.tile([128, KO, 128], BF16, tag="wq1", name="wq1_sb")
            wq2_sb = persist.tile([128, KO, 128], BF16, tag="wq2", name="wq2_sb")
            wk1_sb = persist.tile([128, KO, 128], BF16, tag="wk1", name="wk1_sb")
            wk2_sb = persist.tile([128, KO, 128], BF16, tag="wk2", name="wk2_sb")
            wv_sb = persist.tile([128, KO, DG], BF16, tag="wv", name="wv_sb")
            wo_sb = persist.tile([128, 2, D], BF16, tag="wo", name="wo_sb")

            # preload order tuned for the m=0 critical path: Q weights,
            # slice-0 rope tables, K weights, V weights, then the rest
            wq1r = wq1.rearrange("(ko p) m -> p ko m", p=128)
            nc.scalar.dma_start(wq1_sb[:, 0:2], wq1r[:, 0:2])
            nc.scalar.dma_start(wq1_sb[:, 2:8], wq1r[:, 2:8])
            nc.scalar.dma_start(
                wq2_sb[:], wq2.rearrange("(ko p) m -> p ko m", p=128))
            nc.scalar.dma_start(ccss_sb[:, :, 0:512], ccss[:, :, 0:512])
            nc.gpsimd.dma_start(sscc_sb[:, :, 0:512], sscc[:, :, 0:512])
            nc.gpsimd.dma_start(
                wk1_sb[:], wk1.rearrange("(ko p) m -> p ko m", p=128))
            nc.gpsimd.dma_start(
                wk2_sb[:], wk2.rearrange("(ko p) m -> p ko m", p=128))
            nc.scalar.dma_start(
                wv_sb[:], wv.rearrange("(ko p) m -> p ko m", p=128))
            nc.sync.dma_start(id_sb[:], ident[:])
            nc.sync.dma_start(mc_sb[:], maskc[:])
            for ms in range(1, N_SLICES):
                sl = slice(512 * ms, 512 * (ms + 1))
                nc.scalar.dma_start(ccss_sb[:, :, sl], ccss[:, :, sl])
                nc.gpsimd.dma_start(sscc_sb[:, :, sl], sscc[:, :, sl])
            nc.scalar.dma_start(
                wo_sb[:], wo.rearrange("(ko p) m -> p ko m", p=128))

            # ones column of vt
            for ms in range(N_SLICES):
                nc.vector.memset(vt[ms][:, :, :, DH], 1.0)

            if True:
                F32R = mybir.dt.float32r
                onesf = persist.tile([128, 64], F32, tag="onesf", name="onesf")
                ones1 = persist.tile([1, 64], F32R, tag="ones1", name="ones1")
                nc.vector.memset(onesf[:], 1.0)
                nc.vector.tensor_copy(ones1[:], onesf[0:1, :])

            # ---- PSUM pools ----
            pqps = ctx.enter_context(
                tc.tile_pool(name="pqps", bufs=1, space="PSUM"))
            stps = ctx.enter_context(
                tc.tile_pool(name="stps", bufs=2, space="PSUM"))
            atps = ctx.enter_context(
                tc.tile_pool(name="atps", bufs=1, space="PSUM"))

            def rope_proj(m, xts, w1_sb, w2_sb, dst):
                sl = slice(512 * m, 512 * (m + 1))
                if True:
                    pq = pqps.tile([128, 2, 512], F32, tag="pq", name="pq")
                    for half, w_sb in ((0, w1_sb), (1, w2_sb)):
                        for ko in range(KO):
                            nc.tensor.matmul(pq[:, half, :], w_sb[:, ko],
                                             xts[:, ko],
                                             start=(ko == 0), stop=(ko == KO - 1))
                    # rope: t12 = [x1*cos | x2*sin], t43 = [x1*sin | x2*cos]
                    t12 = work.tile([128, 2, 512], BF16, tag="t12", name="t12")
                    t43 = work.tile([128, 2, 512], BF16, tag="t43", name="t43")
                    nc.vector.tensor_mul(t12[:], pq[:], ccss_sb[:, :, sl])
                    nc.vector.tensor_mul(t43[:], pq[:], sscc_sb[:, :, sl])
                    for h in range(HPC):
                        pr, a = h // 2, h % 2
                        hs = slice(32 * h, 32 * h + 32)
                        nc.vector.tensor_sub(
                            dst[pr][m][64 * a:64 * a + 32, :],
                            t12[hs, 0], t12[hs, 1])
                        nc.vector.tensor_add(
                            dst[pr][m][64 * a + 32:64 * a + 64, :],
                            t43[hs, 1], t43[hs, 0])

            def proj_q(m):
                """x slice DMA + Q projection + RoPE for slice m."""
                xts = work.tile([128, KO, 512], BF16, tag="xts", name="xts")
                xTr = xT.rearrange("(ko p) s -> p ko s", p=128)
                sl = slice(512 * m, 512 * (m + 1))
                nc.sync.dma_start(xts[:, 0:2], xTr[:, 0:2, sl])
                nc.sync.dma_start(xts[:, 2:5], xTr[:, 2:5, sl])
                nc.sync.dma_start(xts[:, 5:8], xTr[:, 5:8, sl])
                rope_proj(m, xts, wq1_sb, wq2_sb, qr)
                return xts

            def proj_kv(m, xts):
                """K projection + RoPE, V projection for slice m."""
                rope_proj(m, xts, wk1_sb, wk2_sb, kr)
                for scp in range(2):
                    pv = pqps.tile([128, 2, 512], F32, tag="pq", name="pv")
                    for sc2 in range(2):
                        sc = 2 * scp + sc2
                        for ko in range(KO):
                            nc.tensor.matmul(
                                pv[:, sc2, 0:DG],
                                xts[:, ko, 128 * sc:128 * sc + 128],
                                wv_sb[:, ko],
                                start=(ko == 0), stop=(ko == KO - 1))
                    nc.scalar.copy(
                        vt[m][:, 2 * scp:2 * scp + 2, :, 0:DH],
                        pv[:, :, 0:DG].rearrange("p s (h d) -> p s h d", d=DH))

            def score_unit(m, p, kb):
                """Emit scores + mask inject + exp for (m, p, kb); return the
                bf16 probs tile and the slice written."""
                km, j = kb // 4, kb % 4
                ksl = slice(128 * j, 128 * j + 128)
                diag = (km == m)
                c0 = 128 * j if diag else 0
                st = stps.tile([128, 2, 512], F32, tag="st", name="st")
                for a in range(2):
                    nc.tensor.matmul(
                        st[:, a, c0:],
                        kr[p][km][64 * a:64 * a + 64, ksl],
                        qr[p][m][64 * a:64 * a + 64, c0:],
                        start=True, stop=not diag)
                if diag:
                    for a in range(2):
                        nc.tensor.matmul(
                            st[:, a, c0:c0 + 128], id_sb[:], mc_sb[:],
                            start=False, stop=True,
                            skip_group_check=True)
                pt = work.tile([128, 2, 512], BF16, tag="pt",
                               name="pt", bufs=6)
                nc.scalar.activation(
                    pt[:, :, c0:], st[:, :, c0:], Exp, scale=EXP_SCALE)
                return pt, c0

            def pv_unit(m, p, kb, n_kb, at, pt, c0):
                km, j = kb // 4, kb % 4
                for a in range(2):
                    nc.tensor.matmul(
                        at[:, a, c0:], vt[km][:, j, 2 * p + a],
                        pt[:, a, c0:],
                        start=(kb == 0), stop=(kb == n_kb - 1))

            def attention(m):
                # software-pipelined across both p-chains: scores/exp run one
                # unit ahead of PV so the PE never head-of-line blocks on exp
                n_kb = 4 * m + 4
                units = [(p, kb) for p in range(2) for kb in range(n_kb)]
                ats = {}
                pending = []
                normalized = []

                def do_normalize(p, at):
                    normalize(m, p, at, last=(m == 3 and p == 1))
                    normalized.append(p)

                for i, (p, kb) in enumerate(units):
                    if kb == 0:
                        ats[p] = atps.tile([DH + 1, 2, 512], F32, tag="at",
                                           name="at")
                    pt, c0 = score_unit(m, p, kb)
                    pending.append((p, kb, pt, c0))
                    if len(pending) > 3:
                        pp, pkb, ppt, pc0 = pending.pop(0)
                        pv_unit(m, pp, pkb, n_kb, ats[pp], ppt, pc0)
                        if pkb == n_kb - 1:
                            do_normalize(pp, ats[pp])
                while pending:
                    pp, pkb, ppt, pc0 = pending.pop(0)
                    pv_unit(m, pp, pkb, n_kb, ats[pp], ppt, pc0)
                    if pkb == n_kb - 1:
                        do_normalize(pp, ats[pp])
            def normalize(m, p, at, last=False):
                # atn = at[0:64] * recip(denominator row)
                if USE_GPSIMD_BCAST and not last:
                    dsum = work.tile([1, 2, 512], F32, tag="dsum", name="dsum")
                    nc.vector.tensor_copy(dsum[:], at[DH:DH + 1])
                    dbc = work.tile([64, 2, 512], F32, tag="dbc", name="dbc")
                    nc.gpsimd.partition_broadcast(dbc[:], dsum[:])
                    rbc = work.tile([64, 2, 512], F32, tag="rbc", name="rbc")
                    nc.vector.reciprocal_approx_fast(rbc[:], dbc[:])
                    for a in range(2):
                        nc.vector.tensor_mul(
                            atn[p][m][64 * a:64 * a + 64, :],
                            at[0:DH, a], rbc[:, a])
                else:
                    F32R = mybir.dt.float32r
                    for a in range(2):
                        ssum = work.tile([1, 512], F32R, tag="ssum",
                                         name="ssum")
                        nc.vector.tensor_copy(ssum[:], at[DH:DH + 1, a])
                        sbc = stps.tile([64, 512], F32, tag="st",
                                        name="sbc")
                        nc.tensor.matmul(sbc[:], ones1[:], ssum[:],
                                         start=True, stop=True)
                        rbc = work.tile([64, 512], F32, tag="rbc",
                                        name="rbc")
                        nc.vector.reciprocal_approx_fast(rbc[:], sbc[:])
                        nc.vector.tensor_mul(
                            atn[p][m][64 * a:64 * a + 64, :],
                            at[0:DH, a], rbc[:])

            def outproj(m):
                for sc in range(4 * m, 4 * m + 4):
                    scl = slice(128 * (sc % 4), 128 * (sc % 4) + 128)
                    if m == 3 and sc % 2 == 1:
                        # tail: attention is drained, st slots are free --
                        # ping-pong po between pools so osb copies overlap MMs
                        po = stps.tile([128, 2, 512], F32, tag="st", name="po")
                    else:
                        po = pqps.tile([128, 2, 512], F32, tag="pq", name="po")
                    for ks in range(2):
                        for nh in range(2):
                            nc.tensor.matmul(
                                po[:, nh, :], atn[ks][sc // 4][:, scl],
                                wo_sb[:, ks, 512 * nh:512 * nh + 512],
                                start=(ks == 0), stop=(ks == 1))
                    osb = work.tile([128, 1024], BF16, tag="osb", name="osb")
                    nc.scalar.copy(
                        osb[:].rearrange("p (x n) -> p x n", x=2), po[:])
                    nc.sync.dma_start(o_part[128 * sc:128 * sc + 128, :], osb[:])

            xts0 = proj_q(0)
            proj_kv(0, xts0)
            xts_next = proj_q(1)
            attention(0)
            proj_kv(1, xts_next)
            outproj(0)
            xts_next = proj_q(2)
            attention(1)
            proj_kv(2, xts_next)
            outproj(1)
            xts_next = proj_q(3)
            attention(2)
            proj_kv(3, xts_next)
            outproj(2)
            attention(3)
            outproj(3)

    nc.finalize()
    return nc


def prep_inputs(hidden_states, cos, sin, attention_mask, Wq, Wk, Wv, Wo):
    """Host-side sharding/layout prep. Returns in_maps for the 8 cores."""
    bf = ml_dtypes.bfloat16
    hs = np.asarray(hidden_states, dtype=np.float32)
    cos = np.asarray(cos, dtype=np.float32)
    sin = np.asarray(sin, dtype=np.float32)
    Wq = np.asarray(Wq, dtype=np.float32)
    Wk = np.asarray(Wk, dtype=np.float32)
    Wv = np.asarray(Wv, dtype=np.float32)
    Wo = np.asarray(Wo, dtype=np.float32)

    # ccss[p, 0, s] = cos[s, p%32]; ccss[p, 1, s] = sin[s, p%32]
    ct = np.tile(cos.T, (4, 1))  # [128, S]
    st_ = np.tile(sin.T, (4, 1))
    ccss = np.ascontiguousarray(np.stack([ct, st_], axis=1).astype(bf))
    sscc = np.ascontiguousarray(np.stack([st_, ct], axis=1).astype(bf))

    idm = np.eye(128, dtype=bf)
    kappa = np.arange(128)[:, None]
    u = np.arange(128)[None, :]
    maskc = np.where(u >= kappa, 0.0, MASK_VAL).astype(bf)

    xTs = [np.ascontiguousarray(hs[b].T.astype(bf)) for b in range(B)]

    in_maps = []
    for c in range(N_CORES):
        b, g = c // 4, c % 4
        hsl = slice(DG * g, DG * (g + 1))
        wq_g = Wq[:, hsl].reshape(D, HPC, DH)
        wk_g = Wk[:, hsl].reshape(D, HPC, DH)
        in_maps.append({
            "xT": xTs[b],
            "wq1": np.ascontiguousarray(
                wq_g[:, :, :32].reshape(D, 128).astype(bf)),
            "wq2": np.ascontiguousarray(
                wq_g[:, :, 32:].reshape(D, 128).astype(bf)),
            "wk1": np.ascontiguousarray(
                wk_g[:, :, :32].reshape(D, 128).astype(bf)),
            "wk2": np.ascontiguousarray(
                wk_g[:, :, 32:].reshape(D, 128).astype(bf)),
            "wv": np.ascontiguousarray(Wv[:, hsl].astype(bf)),
            "wo": np.ascontiguousarray(Wo[hsl, :].astype(bf)),
            "ccss": ccss,
            "sscc": sscc,
            "ident": idm,
            "maskc": maskc,
        })
    return in_maps


_NC_CACHE = {}


def get_nc():
    if "nc" not in _NC_CACHE:
        _NC_CACHE["nc"] = build_nc()
    return _NC_CACHE["nc"]


def run(inputs, trace=False):
    """Returns (output [B,S,D] fp32, BassKernelResults)."""
    nc = get_nc()
    in_maps = prep_inputs(**inputs)
    res = run_bass_kernel_spmd(nc, in_maps, list(range(N_CORES)), trace=trace)
    out = np.zeros((B, S, D), dtype=np.float32)
    for c in range(N_CORES):
        out[c // 4] += res.results[c]["o_part"].astype(np.float32)
    return out, res


def kernel(**inputs):
    return run(inputs, trace=False)[0]


# revision 30
# speedup vs baseline: 1.0354x; 1.0350x over previous
"""Trainium2 Bass kernel for nn_MultiHeadAttention_65987877535893.

MHA with RoPE: B=2, S=2048, D=1024, H=16, Dh=64, causal mask.

Sharding (8 cores): data-parallel over B (x2) x tensor-parallel over heads
(x4 -> 4 heads/core).  Each core computes, for its batch b and head group g:
  QKV projections (column-sharded W), RoPE, causal attention, and a partial
  output  A_g @ Wo_g  (row-sharded Wo).  Host sums the 4 partials per batch.

Final design (all matmul inputs bf16, fp32 PSUM accumulation):
  - projections: lhsT=W chunks (bf16, FWL), rhs = xT slices; Q/K into a
    combined [128,2,512] PSUM tile (x1 dims half 0, x2 dims half 1).
  - RoPE on DVE: 2 fused muls against [cc|ss] / [ss|cc] tables, then 8
    strided sub/add producing head-contiguous bf16 qr/kr tiles.
  - scores TRANSPOSED (ST[k,q] = K @ Q^T), two heads concurrently via PE
    row-tiling (K=64 each, base partitions 0/64); causal mask handled by
    (a) skipping out-of-range blocks, (b) accumulating a constant -3200
    triangular tile into the 128-wide diagonal boundary via an extra
    identity-lhsT matmul (so exp gives exact zeros) -- no DVE mask pass.
  - exp on ScalarE (scale=1/8 folded), bf16 probs.
  - PV: lhsT=[V_h | ones] [128,65] bf16 -> A^T[64,q] + softmax denominator.
  - normalize: reciprocal on DVE, partition-broadcast on GpSimd (attn
    ucode library), two DVE muls -> bf16 atn.
  - output: O = atn^T @ Wo in PSUM, copied to SBUF bf16 on ScalarE,
    DMA'd out as bf16 partials; host sums in fp32.
  PSUM budget: st(tag x2 bufs = 4 banks) + at(2) + pq(2) = 8 banks.
  Scheduling: attention is emitted software-pipelined (scores/exp run 3
  units ahead of PV) across both head-pair chains; Q-proj of slice m+1 is
  hoisted before attention(m) so chain boundaries never wait on RoPE; the
  tail out-projection ping-pongs PSUM slots with the drained score pool.
  Engine balance: PE ~matmuls, ScalarE = exp + PSUM->SBUF copies,
  VectorE = RoPE + normalize muls, GpSimd = denominator broadcast.
"""

import os
import sys

sys.path.insert(0, "/opt/trn_rl_repo")
os.environ.setdefault("MYCRO_LOCAL_CACHE", "1")

import numpy as np
import ml_dtypes

import concourse.bass as bass
import concourse.bacc as bacc
import concourse.mybir as mybir
import concourse.tile as tile
from concourse import library_config
from concourse.bass_utils import run_bass_kernel_spmd

F32 = mybir.dt.float32
BF16 = mybir.dt.bfloat16

B, S, D = 2, 2048, 1024
H = 16
DH = 64
HPC = 4           # heads per core
DG = HPC * DH     # 256
N_CORES = 8
KO = D // 128     # 8 contraction chunks
N_SLICES = S // 512   # 4 q/s slices
EXP_SCALE = float(DH) ** -0.5  # 0.125
MASK_VAL = -3200.0
Exp = mybir.ActivationFunctionType.Exp

USE_GPSIMD_BCAST = True


def build_nc():
    nc = bacc.Bacc()

    xT = nc.dram_tensor("xT", [D, S], BF16, kind="ExternalInput")
    wq1 = nc.dram_tensor("wq1", [D, 128], BF16, kind="ExternalInput")
    wq2 = nc.dram_tensor("wq2", [D, 128], BF16, kind="ExternalInput")
    wk1 = nc.dram_tensor("wk1", [D, 128], BF16, kind="ExternalInput")
    wk2 = nc.dram_tensor("wk2", [D, 128], BF16, kind="ExternalInput")
    wv = nc.dram_tensor("wv", [D, DG], BF16, kind="ExternalInput")
    wo = nc.dram_tensor("wo", [DG, D], BF16, kind="ExternalInput")
    ccss = nc.dram_tensor("ccss", [128, 2, S], BF16, kind="ExternalInput")
    sscc = nc.dram_tensor("sscc", [128, 2, S], BF16, kind="ExternalInput")
    ident = nc.dram_tensor("ident", [128, 128], BF16, kind="ExternalInput")
    maskc = nc.dram_tensor("maskc", [128, 128], BF16, kind="ExternalInput")
    o_part = nc.dram_tensor("o_part", [S, D], BF16, kind="ExternalOutput")

    with tile.TileContext(nc) as tc:
        import contextlib

        ctx = contextlib.ExitStack()
        with ctx:
            if USE_GPSIMD_BCAST:
                nc.gpsimd.load_library(library_config.attn)

            persist = ctx.enter_context(tc.tile_pool(name="persist", bufs=1))
            work = ctx.enter_context(tc.tile_pool(name="work", bufs=2))

            # ---- persistent SBUF tensors ----
            qr = [[persist.tile([128, 512], BF16, tag=f"qr{p}_{ms}", name=f"qr{p}_{ms}")
                   for ms in range(N_SLICES)] for p in range(2)]
            kr = [[persist.tile([128, 512], BF16, tag=f"kr{p}_{ms}", name=f"kr{p}_{ms}")
                   for ms in range(N_SLICES)] for p in range(2)]
            # V with a ones column per (kb, head): [128, kb_in_slice, head, 65]
            vt = [persist.tile([128, 4, HPC, DH + 1], BF16, tag=f"vt{ms}", name=f"vt{ms}")
                  for ms in range(N_SLICES)]
            atn = [[persist.tile([128, 512], BF16, tag=f"atn{p}_{ms}", name=f"atn{p}_{ms}")
                    for ms in range(N_SLICES)] for p in range(2)]
            ccss_sb = persist.tile([128, 2, S], BF16, tag="ccss", name="ccss_sb")
            sscc_sb = persist.tile([128, 2, S], BF16, tag="sscc", name="sscc_sb")
            id_sb = persist.tile([128, 128], BF16, tag="id", name="id_sb")
            mc_sb = persist.tile([128, 128], BF16, tag="mc", name="mc_sb")
            wq1_sb = persist.tile([128, KO, 128], BF16, tag="wq1", name="wq1_sb")
            wq2_sb = persist.tile([128, KO, 128], BF16, tag="wq2", name="wq2_sb")
            wk1_sb = persist.tile([128, KO, 128], BF16, tag="wk1", name="wk1_sb")
            wk2_sb = persist.tile([128, KO, 128], BF16, tag="wk2", name="wk2_sb")
            wv_sb = persist.tile([128, KO, DG], BF16, tag="wv", name="wv_sb")
            wo_sb = persist.tile([128, 2, D], BF16, tag="wo", name="wo_sb")

            # preload order tuned for the m=0 critical path: Q weights,
            # slice-0 rope tables, K weights, V weights, then the rest
            wq1r = wq1.rearrange("(ko p) m -> p ko m", p=128)
            nc.scalar.dma_start(wq1_sb[:, 0:2], wq1r[:, 0:2])
            nc.scalar.dma_start(wq1_sb[:, 2:8], wq1r[:, 2:8])
            nc.scalar.dma_start(
                wq2_sb[:], wq2.rearrange("(ko p) m -> p ko m", p=128))
            nc.scalar.dma_start(ccss_sb[:, :, 0:512], ccss[:, :, 0:512])
            nc.gpsimd.dma_start(sscc_sb[:, :, 0:512], sscc[:, :, 0:512])
            nc.gpsimd.dma_start(
                wk1_sb[:], wk1.rearrange("(ko p) m -> p ko m", p=128))
            nc.gpsimd.dma_start(
                wk2_sb[:], wk2.rearrange("(ko p) m -> p ko m", p=128))
            nc.scalar.dma_start(
                wv_sb[:], wv.rearrange("(ko p) m -> p ko m", p=128))
            nc.sync.dma_start(id_sb[:], ident[:])
            nc.sync.dma_start(mc_sb[:], maskc[:])
            for ms in range(1, N_SLICES):
                sl = slice(512 * ms, 512 * (ms + 1))
                nc.scalar.dma_start(ccss_sb[:, :, sl], ccss[:, :, sl])
                nc.gpsimd.dma_start(sscc_sb[:, :, sl], sscc[:, :, sl])
            nc.scalar.dma_start(
                wo_sb[:], wo.rearrange("(ko p) m -> p ko m", p=128))

            # ones column of vt
            for ms in range(N_SLICES):
                nc.vector.memset(vt[ms][:, :, :, DH], 1.0)

            if True:
                F32R = mybir.dt.float32r
                onesf = persist.tile([128, 64], F32, tag="onesf", name="onesf")
                ones1 = persist.tile([1, 64], F32R, tag="ones1", name="ones1")
                nc.vector.memset(onesf[:], 1.0)
                nc.vector.tensor_copy(ones1[:], onesf[0:1, :])

            # ---- PSUM pools ----
            pqps = ctx.enter_context(
                tc.tile_pool(name="pqps", bufs=1, space="PSUM"))
            stps = ctx.enter_context(
                tc.tile_pool(name="stps", bufs=2, space="PSUM"))
            atps = ctx.enter_context(
                tc.tile_pool(name="atps", bufs=1, space="PSUM"))

            def rope_proj(m, xts, w1_sb, w2_sb, dst):
                sl = slice(512 * m, 512 * (m + 1))
                if True:
                    pq = pqps.tile([128, 2, 512], F32, tag="pq", name="pq")
                    for half, w_sb in ((0, w1_sb), (1, w2_sb)):
                        for ko in range(KO):
                            nc.tensor.matmul(pq[:, half, :], w_sb[:, ko],
                                             xts[:, ko],
                                             start=(ko == 0), stop=(ko == KO - 1))
                    # rope: t12 = [x1*cos | x2*sin], t43 = [x1*sin | x2*cos]
                    t12 = work.tile([128, 2, 512], BF16, tag="t12", name="t12")
                    t43 = work.tile([128, 2, 512], BF16, tag="t43", name="t43")
                    nc.vector.tensor_mul(t12[:], pq[:], ccss_sb[:, :, sl])
                    nc.vector.tensor_mul(t43[:], pq[:], sscc_sb[:, :, sl])
                    for h in range(HPC):
                        pr, a = h // 2, h % 2
                        hs = slice(32 * h, 32 * h + 32)
                        nc.vector.tensor_sub(
                            dst[pr][m][64 * a:64 * a + 32, :],
                            t12[hs, 0], t12[hs, 1])
                        nc.vector.tensor_add(
                            dst[pr][m][64 * a + 32:64 * a + 64, :],
                            t43[hs, 1], t43[hs, 0])

            def proj_q(m):
                """x slice DMA + Q projection + RoPE for slice m."""
                xts = work.tile([128, KO, 512], BF16, tag="xts", name="xts")
                xTr = xT.rearrange("(ko p) s -> p ko s", p=128)
                sl = slice(512 * m, 512 * (m + 1))
                nc.sync.dma_start(xts[:, 0:2], xTr[:, 0:2, sl])
                nc.sync.dma_start(xts[:, 2:5], xTr[:, 2:5, sl])
                nc.sync.dma_start(xts[:, 5:8], xTr[:, 5:8, sl])
                rope_proj(m, xts, wq1_sb, wq2_sb, qr)
                return xts

            def proj_kv(m, xts):
                """K projection + RoPE, V projection for slice m."""
                rope_proj(m, xts, wk1_sb, wk2_sb, kr)
                for scp in range(2):
                    pv = pqps.tile([128, 2, 512], F32, tag="pq", name="pv")
                    for sc2 in range(2):
                        sc = 2 * scp + sc2
                        for ko in range(KO):
                            nc.tensor.matmul(
                                pv[:, sc2, 0:DG],
                                xts[:, ko, 128 * sc:128 * sc + 128],
                                wv_sb[:, ko],
                                start=(ko == 0), stop=(ko == KO - 1))
                    nc.scalar.copy(
                        vt[m][:, 2 * scp:2 * scp + 2, :, 0:DH],
                        pv[:, :, 0:DG].rearrange("p s (h d) -> p s h d", d=DH))

            def score_unit(m, p, kb):
                """Emit scores + mask inject + exp for (m, p, kb); return the
                bf16 probs tile and the slice written."""
                km, j = kb // 4, kb % 4
                ksl = slice(128 * j, 128 * j + 128)
                diag = (km == m)
                c0 = 128 * j if diag else 0
                st = stps.tile([128, 2, 512], F32, tag="st", name="st")
                for a in range(2):
                    nc.tensor.matmul(
                        st[:, a, c0:],
                        kr[p][km][64 * a:64 * a + 64, ksl],
                        qr[p][m][64 * a:64 * a + 64, c0:],
                        start=True, stop=not diag)
                if diag:
                    for a in range(2):
                        nc.tensor.matmul(
                            st[:, a, c0:c0 + 128], id_sb[:], mc_sb[:],
                            start=False, stop=True,
                            skip_group_check=True)
                pt = work.tile([128, 2, 512], BF16, tag="pt",
                               name="pt", bufs=6)
                nc.scalar.activation(
                    pt[:, :, c0:], st[:, :, c0:], Exp, scale=EXP_SCALE)
                return pt, c0

            def pv_unit(m, p, kb, n_kb, at, pt, c0):
                km, j = kb // 4, kb % 4
                for a in range(2):
                    nc.tensor.matmul(
                        at[:, a, c0:], vt[km][:, j, 2 * p + a],
                        pt[:, a, c0:],
                        start=(kb == 0), stop=(kb == n_kb - 1))

            def attention(m):
                # software-pipelined across both p-chains: scores/exp run one
                # unit ahead of PV so the PE never head-of-line blocks on exp
                n_kb = 4 * m + 4
                units = [(p, kb) for p in range(2) for kb in range(n_kb)]
                ats = {}
                pending = []
                normalized = []

                def do_normalize(p, at):
                    normalize(m, p, at, last=(m == 3 and p == 1))
                    normalized.append(p)

                for i, (p, kb) in enumerate(units):
                    if kb == 0:
                        ats[p] = atps.tile([DH + 1, 2, 512], F32, tag="at",
                                           name="at")
                    pt, c0 = score_unit(m, p, kb)
                    pending.append((p, kb, pt, c0))
                    if len(pending) > 3:
                        pp, pkb, ppt, pc0 = pending.pop(0)
                        pv_unit(m, pp, pkb, n_kb, ats[pp], ppt, pc0)
                        if pkb == n_kb - 1:
                            do_normalize(pp, ats[pp])
                while pending:
                    pp, pkb, ppt, pc0 = pending.pop(0)
                    pv_unit(m, pp, pkb, n_kb, ats[pp], ppt, pc0)
                    if pkb == n_kb - 1:
                        do_normalize(pp, ats[pp])
            def normalize(m, p, at, last=False):
                # atn = at[0:64] * recip(denominator row)
                if USE_GPSIMD_BCAST and not last:
                    dsum = work.tile([1, 2, 512], F32, tag="dsum", name="dsum")
                    nc.vector.tensor_copy(dsum[:], at[DH:DH + 1])
                    dbc = work.tile([64, 2, 512], F32, tag="dbc", name="dbc")
                    nc.gpsimd.partition_broadcast(dbc[:], dsum[:])
                    rbc = work.tile([64, 2, 512], F32, tag="rbc", name="rbc")
                    nc.vector.reciprocal_approx_fast(rbc[:], dbc[:])
                    for a in range(2):
                        nc.vector.tensor_mul(
                            atn[p][m][64 * a:64 * a + 64, :],
                            at[0:DH, a], rbc[:, a])
                else:
                    F32R = mybir.dt.float32r
                    for a in range(2):
                        ssum = work.tile([1, 512], F32R, tag="ssum",
                                         name="ssum")
                        nc.vector.tensor_copy(ssum[:], at[DH:DH + 1, a])
                        sbc = stps.tile([64, 512], F32, tag="st",
                                        name="sbc")
                        nc.tensor.matmul(sbc[:], ones1[:], ssum[:],
                                         start=True, stop=True)
                        rbc = work.tile([64, 512], F32, tag="rbc",
                                        name="rbc")
                        nc.vector.reciprocal_approx_fast(rbc[:], sbc[:])
                        nc.vector.tensor_mul(
                            atn[p][m][64 * a:64 * a + 64, :],
                            at[0:DH, a], rbc[:])

            def outproj(m):
                for sc in range(4 * m, 4 * m + 4):
                    scl = slice(128 * (sc % 4), 128 * (sc % 4) + 128)
                    if m == 3 and sc % 2 == 1:
                        # tail: attention is drained, st slots are free --
                        # ping-pong po between pools so osb copies overlap MMs
                        po = stps.tile([128, 2, 512], F32, tag="st", name="po")
                    else:
                        po = pqps.tile([128, 2, 512], F32, tag="pq", name="po")
                    for ks in range(2):
                        for nh in range(2):
                            nc.tensor.matmul(
                                po[:, nh, :], atn[ks][sc // 4][:, scl],
                                wo_sb[:, ks, 512 * nh:512 * nh + 512],
                                start=(ks == 0), stop=(ks == 1))
                    osb = work.tile([128, 1024], BF16, tag="osb", name="osb")
                    nc.scalar.copy(
                        osb[:].rearrange("p (x n) -> p x n", x=2), po[:])
                    nc.sync.dma_start(o_part[128 * sc:128 * sc + 128, :], osb[:])

            xts0 = proj_q(0)
            proj_kv(0, xts0)
            xts_next = proj_q(1)
            attention(0)
            proj_kv(1, xts_next)
            outproj(0)
            xts_next = proj_q(2)
            attention(1)
            proj_kv(2, xts_next)
            outproj(1)
            xts_next = proj_q(3)
            attention(2)
            proj_kv(3, xts_next)
            outproj(2)
            attention(3)
            outproj(3)

    nc.finalize()
    return nc


def prep_inputs(hidden_states, cos, sin, attention_mask, Wq, Wk, Wv, Wo):
    """Host-side sharding/layout prep. Returns in_maps for the 8 cores."""
    bf = ml_dtypes.bfloat16
    hs = np.asarray(hidden_states, dtype=np.float32)
    cos = np.asarray(cos, dtype=np.float32)
    sin = np.asarray(sin, dtype=np.float32)
    Wq = np.asarray(Wq, dtype=np.float32)
    Wk = np.asarray(Wk, dtype=np.float32)
    Wv = np.asarray(Wv, dtype=np.float32)
    Wo = np.asarray(Wo, dtype=np.float32)

    # ccss[p, 0, s] = cos[s, p%32]; ccss[p, 1, s] = sin[s, p%32]
    ct = np.tile(cos.T, (4, 1))  # [128, S]
    st_ = np.tile(sin.T, (4, 1))
    ccss = np.ascontiguousarray(np.stack([ct, st_], axis=1).astype(bf))
    sscc = np.ascontiguousarray(np.stack([st_, ct], axis=1).astype(bf))

    idm = np.eye(128, dtype=bf)
    kappa = np.arange(128)[:, None]
    u = np.arange(128)[None, :]
    maskc = np.where(u >= kappa, 0.0, MASK_VAL).astype(bf)

    xTs = [np.ascontiguousarray(hs[b].T.astype(bf)) for b in range(B)]

    in_maps = []
    for c in range(N_CORES):
        b, g = c // 4, c % 4
        hsl = slice(DG * g, DG * (g + 1))
        wq_g = Wq[:, hsl].reshape(D, HPC, DH)
        wk_g = Wk[:, hsl].reshape(D, HPC, DH)
        in_maps.append({
            "xT": xTs[b],
            "wq1": np.ascontiguousarray(
                wq_g[:, :, :32].reshape(D, 128).astype(bf)),
            "wq2": np.ascontiguousarray(
                wq_g[:, :, 32:].reshape(D, 128).astype(bf)),
            "wk1": np.ascontiguousarray(
                wk_g[:, :, :32].reshape(D, 128).astype(bf)),
            "wk2": np.ascontiguousarray(
                wk_g[:, :, 32:].reshape(D, 128).astype(bf)),
            "wv": np.ascontiguousarray(Wv[:, hsl].astype(bf)),
            "wo": np.ascontiguousarray(Wo[hsl, :].astype(bf)),
            "ccss": ccss,
            "sscc": sscc,
            "ident": idm,
            "maskc": maskc,
        })
    return in_maps


_NC_CACHE = {}


def get_nc():
    if "nc" not in _NC_CACHE:
        _NC_CACHE["nc"] = build_nc()
    return _NC_CACHE["nc"]


def run(inputs, trace=False):
    """Returns (output [B,S,D] fp32, BassKernelResults)."""
    nc = get_nc()
    in_maps = prep_inputs(**inputs)
    res = run_bass_kernel_spmd(nc, in_maps, list(range(N_CORES)), trace=trace)
    out = np.zeros((B, S, D), dtype=np.float32)
    for c in range(N_CORES):
        out[c // 4] += res.results[c]["o_part"].astype(np.float32)
    return out, res


def kernel(**inputs):
    return run(inputs, trace=False)[0]


# revision 31
# speedup vs baseline: 1.0484x; 1.0126x over previous
"""Trainium2 Bass kernel for nn_MultiHeadAttention_65987877535893.

MHA with RoPE: B=2, S=2048, D=1024, H=16, Dh=64, causal mask.

Sharding (8 cores): data-parallel over B (x2) x tensor-parallel over heads
(x4 -> 4 heads/core).  Each core computes, for its batch b and head group g:
  QKV projections (column-sharded W), RoPE, causal attention, and a partial
  output  A_g @ Wo_g  (row-sharded Wo).  Host sums the 4 partials per batch.

Final design (all matmul inputs bf16, fp32 PSUM accumulation):
  - projections: lhsT=W chunks (bf16, FWL), rhs = xT slices; Q/K into a
    combined [128,2,512] PSUM tile (x1 dims half 0, x2 dims half 1).
  - RoPE on DVE: 2 fused muls against [cc|ss] / [ss|cc] tables, then 8
    strided sub/add producing head-contiguous bf16 qr/kr tiles.
  - scores TRANSPOSED (ST[k,q] = K @ Q^T), two heads concurrently via PE
    row-tiling (K=64 each, base partitions 0/64); causal mask handled by
    (a) skipping out-of-range blocks, (b) accumulating a constant -3200
    triangular tile into the 128-wide diagonal boundary via an extra
    identity-lhsT matmul (so exp gives exact zeros) -- no DVE mask pass.
  - exp on ScalarE (scale=1/8 folded), bf16 probs.
  - PV: lhsT=[V_h | ones] [128,65] bf16 -> A^T[64,q] + softmax denominator.
  - normalize: reciprocal on DVE, partition-broadcast on GpSimd (attn
    ucode library), two DVE muls -> bf16 atn.
  - output: O = atn^T @ Wo in PSUM, copied to SBUF bf16 on ScalarE,
    DMA'd out as bf16 partials; host sums in fp32.
  PSUM budget: st(tag x2 bufs = 4 banks) + at(2) + pq(2) = 8 banks.
  Scheduling: attention is emitted software-pipelined (scores/exp run 3
  units ahead of PV) across both head-pair chains; Q-proj of slice m+1 is
  hoisted before attention(m) so chain boundaries never wait on RoPE; the
  tail out-projection ping-pongs PSUM slots with the drained score pool.
  Engine balance: PE ~matmuls, ScalarE = exp + PSUM->SBUF copies,
  VectorE = RoPE + normalize muls, GpSimd = denominator broadcast.
"""

import os
import sys

sys.path.insert(0, "/opt/trn_rl_repo")
os.environ.setdefault("MYCRO_LOCAL_CACHE", "1")

import numpy as np
import ml_dtypes

import concourse.bass as bass
import concourse.bacc as bacc
import concourse.mybir as mybir
import concourse.tile as tile
from concourse import library_config
from concourse.bass_utils import run_bass_kernel_spmd

F32 = mybir.dt.float32
BF16 = mybir.dt.bfloat16

B, S, D = 2, 2048, 1024
H = 16
DH = 64
HPC = 4           # heads per core
DG = HPC * DH     # 256
N_CORES = 8
KO = D // 128     # 8 contraction chunks
N_SLICES = S // 512   # 4 q/s slices
EXP_SCALE = float(DH) ** -0.5  # 0.125
MASK_VAL = -3200.0
Exp = mybir.ActivationFunctionType.Exp

USE_GPSIMD_BCAST = True


def build_nc():
    nc = bacc.Bacc()

    xT = nc.dram_tensor("xT", [D, S], BF16, kind="ExternalInput")
    wq1 = nc.dram_tensor("wq1", [D, 128], BF16, kind="ExternalInput")
    wq2 = nc.dram_tensor("wq2", [D, 128], BF16, kind="ExternalInput")
    wk1 = nc.dram_tensor("wk1", [D, 128], BF16, kind="ExternalInput")
    wk2 = nc.dram_tensor("wk2", [D, 128], BF16, kind="ExternalInput")
    wv = nc.dram_tensor("wv", [D, DG], BF16, kind="ExternalInput")
    wo = nc.dram_tensor("wo", [DG, D], BF16, kind="ExternalInput")
    ccss = nc.dram_tensor("ccss", [128, 2, S], BF16, kind="ExternalInput")
    sscc = nc.dram_tensor("sscc", [128, 2, S], BF16, kind="ExternalInput")
    ident = nc.dram_tensor("ident", [128, 128], BF16, kind="ExternalInput")
    maskc = nc.dram_tensor("maskc", [128, 128], BF16, kind="ExternalInput")
    o_part = nc.dram_tensor("o_part", [S, D], BF16, kind="ExternalOutput")

    with tile.TileContext(nc) as tc:
        import contextlib

        ctx = contextlib.ExitStack()
        with ctx:
            if USE_GPSIMD_BCAST:
                nc.gpsimd.load_library(library_config.attn)

            persist = ctx.enter_context(tc.tile_pool(name="persist", bufs=1))
            work = ctx.enter_context(tc.tile_pool(name="work", bufs=2))

            # ---- persistent SBUF tensors ----
            qr = [[persist.tile([128, 512], BF16, tag=f"qr{p}_{ms}", name=f"qr{p}_{ms}")
                   for ms in range(N_SLICES)] for p in range(2)]
            kr = [[persist.tile([128, 512], BF16, tag=f"kr{p}_{ms}", name=f"kr{p}_{ms}")
                   for ms in range(N_SLICES)] for p in range(2)]
            # V with a ones column per (kb, head): [128, kb_in_slice, head, 65]
            vt = [persist.tile([128, 4, HPC, DH + 1], BF16, tag=f"vt{ms}", name=f"vt{ms}")
                  for ms in range(N_SLICES)]
            atn = [[persist.tile([128, 512], BF16, tag=f"atn{p}_{ms}", name=f"atn{p}_{ms}")
                    for ms in range(N_SLICES)] for p in range(2)]
            ccss_sb = persist.tile([128, 2, S], BF16, tag="ccss", name="ccss_sb")
            sscc_sb = persist.tile([128, 2, S], BF16, tag="sscc", name="sscc_sb")
            id_sb = persist.tile([128, 128], BF16, tag="id", name="id_sb")
            mc_sb = persist.tile([128, 128], BF16, tag="mc", name="mc_sb")
            wq1_sb = persist.tile([128, KO, 128], BF16, tag="wq1", name="wq1_sb")
            wq2_sb = persist.tile([128, KO, 128], BF16, tag="wq2", name="wq2_sb")
            wk1_sb = persist.tile([128, KO, 128], BF16, tag="wk1", name="wk1_sb")
            wk2_sb = persist.tile([128, KO, 128], BF16, tag="wk2", name="wk2_sb")
            wv_sb = persist.tile([128, KO, DG], BF16, tag="wv", name="wv_sb")
            wo_sb = persist.tile([128, 2, D], BF16, tag="wo", name="wo_sb")

            # preload order tuned for the m=0 critical path: Q weights,
            # slice-0 rope tables, K weights, V weights, then the rest
            wq1r = wq1.rearrange("(ko p) m -> p ko m", p=128)
            nc.scalar.dma_start(wq1_sb[:, 0:2], wq1r[:, 0:2])
            nc.scalar.dma_start(wq1_sb[:, 2:8], wq1r[:, 2:8])
            nc.scalar.dma_start(
                wq2_sb[:], wq2.rearrange("(ko p) m -> p ko m", p=128))
            nc.scalar.dma_start(ccss_sb[:, :, 0:512], ccss[:, :, 0:512])
            nc.gpsimd.dma_start(sscc_sb[:, :, 0:512], sscc[:, :, 0:512])
            nc.gpsimd.dma_start(
                wk1_sb[:], wk1.rearrange("(ko p) m -> p ko m", p=128))
            nc.gpsimd.dma_start(
                wk2_sb[:], wk2.rearrange("(ko p) m -> p ko m", p=128))
            nc.scalar.dma_start(
                wv_sb[:], wv.rearrange("(ko p) m -> p ko m", p=128))
            nc.sync.dma_start(id_sb[:], ident[:])
            nc.sync.dma_start(mc_sb[:], maskc[:])
            for ms in range(1, N_SLICES):
                sl = slice(512 * ms, 512 * (ms + 1))
                nc.scalar.dma_start(ccss_sb[:, :, sl], ccss[:, :, sl])
                nc.gpsimd.dma_start(sscc_sb[:, :, sl], sscc[:, :, sl])
            nc.scalar.dma_start(
                wo_sb[:], wo.rearrange("(ko p) m -> p ko m", p=128))

            # ones column of vt
            for ms in range(N_SLICES):
                nc.vector.memset(vt[ms][:, :, :, DH], 1.0)

            if True:
                F32R = mybir.dt.float32r
                onesf = persist.tile([128, 64], F32, tag="onesf", name="onesf")
                ones1 = persist.tile([1, 64], F32R, tag="ones1", name="ones1")
                nc.vector.memset(onesf[:], 1.0)
                nc.vector.tensor_copy(ones1[:], onesf[0:1, :])

            # ---- PSUM pools ----
            pqps = ctx.enter_context(
                tc.tile_pool(name="pqps", bufs=1, space="PSUM"))
            stps = ctx.enter_context(
                tc.tile_pool(name="stps", bufs=2, space="PSUM"))
            atps = ctx.enter_context(
                tc.tile_pool(name="atps", bufs=1, space="PSUM"))

            def rope_proj(m, xts, w1_sb, w2_sb, dst):
                sl = slice(512 * m, 512 * (m + 1))
                if True:
                    pq = pqps.tile([128, 2, 512], F32, tag="pq", name="pq")
                    for half, w_sb in ((0, w1_sb), (1, w2_sb)):
                        for ko in range(KO):
                            nc.tensor.matmul(pq[:, half, :], w_sb[:, ko],
                                             xts[:, ko],
                                             start=(ko == 0), stop=(ko == KO - 1))
                    # rope: t12 = [x1*cos | x2*sin], t43 = [x1*sin | x2*cos]
                    t12 = work.tile([128, 2, 512], BF16, tag="t12", name="t12")
                    t43 = work.tile([128, 2, 512], BF16, tag="t43", name="t43")
                    nc.vector.tensor_mul(t12[:], pq[:], ccss_sb[:, :, sl])
                    nc.vector.tensor_mul(t43[:], pq[:], sscc_sb[:, :, sl])
                    for h in range(HPC):
                        pr, a = h // 2, h % 2
                        hs = slice(32 * h, 32 * h + 32)
                        nc.vector.tensor_sub(
                            dst[pr][m][64 * a:64 * a + 32, :],
                            t12[hs, 0], t12[hs, 1])
                        nc.vector.tensor_add(
                            dst[pr][m][64 * a + 32:64 * a + 64, :],
                            t43[hs, 1], t43[hs, 0])

            def proj_q(m):
                """x slice DMA + Q projection + RoPE for slice m."""
                xts = work.tile([128, KO, 512], BF16, tag="xts", name="xts")
                xTr = xT.rearrange("(ko p) s -> p ko s", p=128)
                sl = slice(512 * m, 512 * (m + 1))
                nc.sync.dma_start(xts[:, 0:2], xTr[:, 0:2, sl])
                nc.sync.dma_start(xts[:, 2:5], xTr[:, 2:5, sl])
                nc.sync.dma_start(xts[:, 5:8], xTr[:, 5:8, sl])
                rope_proj(m, xts, wq1_sb, wq2_sb, qr)
                return xts

            def proj_kv(m, xts):
                """K projection + RoPE, V projection for slice m."""
                rope_proj(m, xts, wk1_sb, wk2_sb, kr)
                for scp in range(2):
                    pv = pqps.tile([128, 2, 512], F32, tag="pq", name="pv")
                    for sc2 in range(2):
                        sc = 2 * scp + sc2
                        for ko in range(KO):
                            nc.tensor.matmul(
                                pv[:, sc2, 0:DG],
                                xts[:, ko, 128 * sc:128 * sc + 128],
                                wv_sb[:, ko],
                                start=(ko == 0), stop=(ko == KO - 1))
                    nc.scalar.copy(
                        vt[m][:, 2 * scp:2 * scp + 2, :, 0:DH],
                        pv[:, :, 0:DG].rearrange("p s (h d) -> p s h d", d=DH))

            def score_unit(m, p, kb):
                """Emit scores + mask inject + exp for (m, p, kb); return the
                bf16 probs tile and the slice written."""
                km, j = kb // 4, kb % 4
                ksl = slice(128 * j, 128 * j + 128)
                diag = (km == m)
                c0 = 128 * j if diag else 0
                st = stps.tile([128, 2, 512], F32, tag="st", name="st")
                for a in range(2):
                    nc.tensor.matmul(
                        st[:, a, c0:],
                        kr[p][km][64 * a:64 * a + 64, ksl],
                        qr[p][m][64 * a:64 * a + 64, c0:],
                        start=True, stop=not diag)
                if diag:
                    for a in range(2):
                        nc.tensor.matmul(
                            st[:, a, c0:c0 + 128], id_sb[:], mc_sb[:],
                            start=False, stop=True,
                            skip_group_check=True)
                pt = work.tile([128, 2, 512], BF16, tag="pt",
                               name="pt", bufs=6)
                nc.scalar.activation(
                    pt[:, :, c0:], st[:, :, c0:], Exp, scale=EXP_SCALE)
                return pt, c0

            def pv_unit(m, p, kb, n_kb, at, pt, c0):
                km, j = kb // 4, kb % 4
                for a in range(2):
                    nc.tensor.matmul(
                        at[:, a, c0:], vt[km][:, j, 2 * p + a],
                        pt[:, a, c0:],
                        start=(kb == 0), stop=(kb == n_kb - 1))

            def attention(m):
                # software-pipelined across both p-chains: scores/exp run one
                # unit ahead of PV so the PE never head-of-line blocks on exp
                n_kb = 4 * m + 4
                units = [(p, kb) for p in range(2) for kb in range(n_kb)]
                ats = {}
                pending = []
                normalized = []

                def do_normalize(p, at):
                    normalize(m, p, at, last=(m == 3 and p == 1))
                    normalized.append(p)

                for i, (p, kb) in enumerate(units):
                    if kb == 0:
                        ats[p] = atps.tile([DH + 1, 2, 512], F32, tag="at",
                                           name="at")
                    pt, c0 = score_unit(m, p, kb)
                    pending.append((p, kb, pt, c0))
                    if len(pending) > 3:
                        pp, pkb, ppt, pc0 = pending.pop(0)
                        pv_unit(m, pp, pkb, n_kb, ats[pp], ppt, pc0)
                        if pkb == n_kb - 1:
                            do_normalize(pp, ats[pp])
                while pending:
                    pp, pkb, ppt, pc0 = pending.pop(0)
                    pv_unit(m, pp, pkb, n_kb, ats[pp], ppt, pc0)
                    if pkb == n_kb - 1:
                        do_normalize(pp, ats[pp])
            def normalize(m, p, at, last=False):
                # atn = at[0:64] * recip(denominator row)
                if USE_GPSIMD_BCAST and not last:
                    dsum = work.tile([1, 2, 512], F32, tag="dsum", name="dsum")
                    nc.vector.tensor_copy(dsum[:], at[DH:DH + 1])
                    dbc = work.tile([64, 2, 512], F32, tag="dbc", name="dbc")
                    nc.gpsimd.partition_broadcast(dbc[:], dsum[:])
                    rbc = work.tile([64, 2, 512], F32, tag="rbc", name="rbc")
                    nc.vector.reciprocal_approx_fast(rbc[:], dbc[:])
                    for a in range(2):
                        nc.vector.tensor_mul(
                            atn[p][m][64 * a:64 * a + 64, :],
                            at[0:DH, a], rbc[:, a])
                else:
                    F32R = mybir.dt.float32r
                    for a in range(2):
                        ssum = work.tile([1, 512], F32R, tag="ssum",
                                         name="ssum")
                        nc.vector.tensor_copy(ssum[:], at[DH:DH + 1, a])
                        sbc = stps.tile([64, 512], F32, tag="st",
                                        name="sbc")
                        nc.tensor.matmul(sbc[:], ones1[:], ssum[:],
                                         start=True, stop=True)
                        rbc = work.tile([64, 512], F32, tag="rbc",
                                        name="rbc")
                        nc.vector.reciprocal_approx_fast(rbc[:], sbc[:])
                        nc.vector.tensor_mul(
                            atn[p][m][64 * a:64 * a + 64, :],
                            at[0:DH, a], rbc[:])

            def outproj(m):
                for sc in range(4 * m, 4 * m + 4):
                    scl = slice(128 * (sc % 4), 128 * (sc % 4) + 128)
                    if m == 3 and sc % 2 == 1:
                        # tail: attention is drained, st slots are free --
                        # ping-pong po between pools so osb copies overlap MMs
                        po = stps.tile([128, 2, 512], F32, tag="st", name="po")
                    else:
                        po = pqps.tile([128, 2, 512], F32, tag="pq", name="po")
                    for ks in range(2):
                        for nh in range(2):
                            nc.tensor.matmul(
                                po[:, nh, :], atn[ks][sc // 4][:, scl],
                                wo_sb[:, ks, 512 * nh:512 * nh + 512],
                                start=(ks == 0), stop=(ks == 1))
                    osb = work.tile([128, 1024], BF16, tag="osb", name="osb")
                    nc.scalar.copy(
                        osb[:].rearrange("p (x n) -> p x n", x=2), po[:])
                    nc.sync.dma_start(o_part[128 * sc:128 * sc + 128, :], osb[:])

            xts0 = proj_q(0)
            proj_kv(0, xts0)
            xts_next = proj_q(1)
            attention(0)
            proj_kv(1, xts_next)
            xts_next = proj_q(2)
            outproj(0)
            attention(1)
            proj_kv(2, xts_next)
            xts_next = proj_q(3)
            outproj(1)
            attention(2)
            proj_kv(3, xts_next)
            outproj(2)
            attention(3)
            outproj(3)

    nc.finalize()
    return nc


def prep_inputs(hidden_states, cos, sin, attention_mask, Wq, Wk, Wv, Wo):
    """Host-side sharding/layout prep. Returns in_maps for the 8 cores."""
    bf = ml_dtypes.bfloat16
    hs = np.asarray(hidden_states, dtype=np.float32)
    cos = np.asarray(cos, dtype=np.float32)
    sin = np.asarray(sin, dtype=np.float32)
    Wq = np.asarray(Wq, dtype=np.float32)
    Wk = np.asarray(Wk, dtype=np.float32)
    Wv = np.asarray(Wv, dtype=np.float32)
    Wo = np.asarray(Wo, dtype=np.float32)

    # ccss[p, 0, s] = cos[s, p%32]; ccss[p, 1, s] = sin[s, p%32]
    ct = np.tile(cos.T, (4, 1))  # [128, S]
    st_ = np.tile(sin.T, (4, 1))
    ccss = np.ascontiguousarray(np.stack([ct, st_], axis=1).astype(bf))
    sscc = np.ascontiguousarray(np.stack([st_, ct], axis=1).astype(bf))

    idm = np.eye(128, dtype=bf)
    kappa = np.arange(128)[:, None]
    u = np.arange(128)[None, :]
    maskc = np.where(u >= kappa, 0.0, MASK_VAL).astype(bf)

    xTs = [np.ascontiguousarray(hs[b].T.astype(bf)) for b in range(B)]

    in_maps = []
    for c in range(N_CORES):
        b, g = c // 4, c % 4
        hsl = slice(DG * g, DG * (g + 1))
        wq_g = Wq[:, hsl].reshape(D, HPC, DH)
        wk_g = Wk[:, hsl].reshape(D, HPC, DH)
        in_maps.append({
            "xT": xTs[b],
            "wq1": np.ascontiguousarray(
                wq_g[:, :, :32].reshape(D, 128).astype(bf)),
            "wq2": np.ascontiguousarray(
                wq_g[:, :, 32:].reshape(D, 128).astype(bf)),
            "wk1": np.ascontiguousarray(
                wk_g[:, :, :32].reshape(D, 128).astype(bf)),
            "wk2": np.ascontiguousarray(
                wk_g[:, :, 32:].reshape(D, 128).astype(bf)),
            "wv": np.ascontiguousarray(Wv[:, hsl].astype(bf)),
            "wo": np.ascontiguousarray(Wo[hsl, :].astype(bf)),
            "ccss": ccss,
            "sscc": sscc,
            "ident": idm,
            "maskc": maskc,
        })
    return in_maps


_NC_CACHE = {}


def get_nc():
    if "nc" not in _NC_CACHE:
        _NC_CACHE["nc"] = build_nc()
    return _NC_CACHE["nc"]


def run(inputs, trace=False):
    """Returns (output [B,S,D] fp32, BassKernelResults)."""
    nc = get_nc()
    in_maps = prep_inputs(**inputs)
    res = run_bass_kernel_spmd(nc, in_maps, list(range(N_CORES)), trace=trace)
    out = np.zeros((B, S, D), dtype=np.float32)
    for c in range(N_CORES):
        out[c // 4] += res.results[c]["o_part"].astype(np.float32)
    return out, res


def kernel(**inputs):
    return run(inputs, trace=False)[0]


# revision 32
# speedup vs baseline: 1.0562x; 1.0074x over previous
"""Trainium2 Bass kernel for nn_MultiHeadAttention_65987877535893.

MHA with RoPE: B=2, S=2048, D=1024, H=16, Dh=64, causal mask.

Sharding (8 cores): data-parallel over B (x2) x tensor-parallel over heads
(x4 -> 4 heads/core).  Each core computes, for its batch b and head group g:
  QKV projections (column-sharded W), RoPE, causal attention, and a partial
  output  A_g @ Wo_g  (row-sharded Wo).  Host sums the 4 partials per batch.

Final design (all matmul inputs bf16, fp32 PSUM accumulation):
  - projections: lhsT=W chunks (bf16, FWL), rhs = xT slices; Q/K into a
    combined [128,2,512] PSUM tile (x1 dims half 0, x2 dims half 1).
  - RoPE on DVE: 2 fused muls against [cc|ss] / [ss|cc] tables, then 8
    strided sub/add producing head-contiguous bf16 qr/kr tiles.
  - scores TRANSPOSED (ST[k,q] = K @ Q^T), two heads concurrently via PE
    row-tiling (K=64 each, base partitions 0/64); causal mask handled by
    (a) skipping out-of-range blocks, (b) accumulating a constant -3200
    triangular tile into the 128-wide diagonal boundary via an extra
    identity-lhsT matmul (so exp gives exact zeros) -- no DVE mask pass.
  - exp on ScalarE (scale=1/8 folded), bf16 probs.
  - PV: lhsT=[V_h | ones] [128,65] bf16 -> A^T[64,q] + softmax denominator.
  - normalize: reciprocal on DVE, partition-broadcast on GpSimd (attn
    ucode library), two DVE muls -> bf16 atn.
  - output: O = atn^T @ Wo in PSUM, copied to SBUF bf16 on ScalarE,
    DMA'd out as bf16 partials; host sums in fp32.
  PSUM budget: st(tag x2 bufs = 4 banks) + at(2) + pq(2) = 8 banks.
  Scheduling: attention is emitted software-pipelined (scores/exp run 3
  units ahead of PV) across both head-pair chains; Q-proj of slice m+1 is
  hoisted before attention(m) so chain boundaries never wait on RoPE; the
  tail out-projection ping-pongs PSUM slots with the drained score pool.
  Engine balance: PE ~matmuls, ScalarE = exp + PSUM->SBUF copies,
  VectorE = RoPE + normalize muls, GpSimd = denominator broadcast.
"""

import os
import sys

sys.path.insert(0, "/opt/trn_rl_repo")
os.environ.setdefault("MYCRO_LOCAL_CACHE", "1")

import numpy as np
import ml_dtypes

import concourse.bass as bass
import concourse.bacc as bacc
import concourse.mybir as mybir
import concourse.tile as tile
from concourse import library_config
from concourse.bass_utils import run_bass_kernel_spmd

F32 = mybir.dt.float32
BF16 = mybir.dt.bfloat16

B, S, D = 2, 2048, 1024
H = 16
DH = 64
HPC = 4           # heads per core
DG = HPC * DH     # 256
N_CORES = 8
KO = D // 128     # 8 contraction chunks
N_SLICES = S // 512   # 4 q/s slices
EXP_SCALE = float(DH) ** -0.5  # 0.125
MASK_VAL = -3200.0
Exp = mybir.ActivationFunctionType.Exp

USE_GPSIMD_BCAST = True


def build_nc():
    nc = bacc.Bacc()

    xT = nc.dram_tensor("xT", [D, S], BF16, kind="ExternalInput")
    wq1 = nc.dram_tensor("wq1", [D, 128], BF16, kind="ExternalInput")
    wq2 = nc.dram_tensor("wq2", [D, 128], BF16, kind="ExternalInput")
    wk1 = nc.dram_tensor("wk1", [D, 128], BF16, kind="ExternalInput")
    wk2 = nc.dram_tensor("wk2", [D, 128], BF16, kind="ExternalInput")
    wv = nc.dram_tensor("wv", [D, DG], BF16, kind="ExternalInput")
    wo = nc.dram_tensor("wo", [DG, D], BF16, kind="ExternalInput")
    ccss = nc.dram_tensor("ccss", [128, 2, S], BF16, kind="ExternalInput")
    sscc = nc.dram_tensor("sscc", [128, 2, S], BF16, kind="ExternalInput")
    ident = nc.dram_tensor("ident", [128, 128], BF16, kind="ExternalInput")
    maskc = nc.dram_tensor("maskc", [128, 128], BF16, kind="ExternalInput")
    o_part = nc.dram_tensor("o_part", [S, D], BF16, kind="ExternalOutput")

    with tile.TileContext(nc) as tc:
        import contextlib

        ctx = contextlib.ExitStack()
        with ctx:
            if USE_GPSIMD_BCAST:
                nc.gpsimd.load_library(library_config.attn)

            persist = ctx.enter_context(tc.tile_pool(name="persist", bufs=1))
            work = ctx.enter_context(tc.tile_pool(name="work", bufs=2))

            # ---- persistent SBUF tensors ----
            qr = [[persist.tile([128, 512], BF16, tag=f"qr{p}_{ms}", name=f"qr{p}_{ms}")
                   for ms in range(N_SLICES)] for p in range(2)]
            kr = [[persist.tile([128, 512], BF16, tag=f"kr{p}_{ms}", name=f"kr{p}_{ms}")
                   for ms in range(N_SLICES)] for p in range(2)]
            # V with a ones column per (kb, head): [128, kb_in_slice, head, 65]
            vt = [persist.tile([128, 4, HPC, DH + 1], BF16, tag=f"vt{ms}", name=f"vt{ms}")
                  for ms in range(N_SLICES)]
            atn = [[persist.tile([128, 512], BF16, tag=f"atn{p}_{ms}", name=f"atn{p}_{ms}")
                    for ms in range(N_SLICES)] for p in range(2)]
            ccss_sb = persist.tile([128, 2, S], BF16, tag="ccss", name="ccss_sb")
            sscc_sb = persist.tile([128, 2, S], BF16, tag="sscc", name="sscc_sb")
            id_sb = persist.tile([128, 128], BF16, tag="id", name="id_sb")
            mc_sb = persist.tile([128, 128], BF16, tag="mc", name="mc_sb")
            wq1_sb = persist.tile([128, KO, 128], BF16, tag="wq1", name="wq1_sb")
            wq2_sb = persist.tile([128, KO, 128], BF16, tag="wq2", name="wq2_sb")
            wk1_sb = persist.tile([128, KO, 128], BF16, tag="wk1", name="wk1_sb")
            wk2_sb = persist.tile([128, KO, 128], BF16, tag="wk2", name="wk2_sb")
            wv_sb = persist.tile([128, KO, DG], BF16, tag="wv", name="wv_sb")
            wo_sb = persist.tile([128, 2, D], BF16, tag="wo", name="wo_sb")

            # preload order tuned for the m=0 critical path: Q weights,
            # slice-0 rope tables, K weights, V weights, then the rest
            wq1r = wq1.rearrange("(ko p) m -> p ko m", p=128)
            nc.scalar.dma_start(wq1_sb[:, 0:2], wq1r[:, 0:2])
            nc.scalar.dma_start(wq1_sb[:, 2:8], wq1r[:, 2:8])
            nc.scalar.dma_start(
                wq2_sb[:], wq2.rearrange("(ko p) m -> p ko m", p=128))
            nc.scalar.dma_start(ccss_sb[:, :, 0:512], ccss[:, :, 0:512])
            nc.gpsimd.dma_start(sscc_sb[:, :, 0:512], sscc[:, :, 0:512])
            nc.gpsimd.dma_start(
                wk1_sb[:], wk1.rearrange("(ko p) m -> p ko m", p=128))
            nc.gpsimd.dma_start(
                wk2_sb[:], wk2.rearrange("(ko p) m -> p ko m", p=128))
            nc.scalar.dma_start(
                wv_sb[:], wv.rearrange("(ko p) m -> p ko m", p=128))
            nc.sync.dma_start(id_sb[:], ident[:])
            nc.sync.dma_start(mc_sb[:], maskc[:])
            for ms in range(1, N_SLICES):
                sl = slice(512 * ms, 512 * (ms + 1))
                nc.scalar.dma_start(ccss_sb[:, :, sl], ccss[:, :, sl])
                nc.gpsimd.dma_start(sscc_sb[:, :, sl], sscc[:, :, sl])
            nc.scalar.dma_start(
                wo_sb[:], wo.rearrange("(ko p) m -> p ko m", p=128))

            # ones column of vt
            for ms in range(N_SLICES):
                nc.vector.memset(vt[ms][:, :, :, DH], 1.0)

            if True:
                F32R = mybir.dt.float32r
                onesf = persist.tile([128, 64], F32, tag="onesf", name="onesf")
                ones1 = persist.tile([1, 64], F32R, tag="ones1", name="ones1")
                nc.vector.memset(onesf[:], 1.0)
                nc.vector.tensor_copy(ones1[:], onesf[0:1, :])

            # ---- PSUM pools ----
            pqps = ctx.enter_context(
                tc.tile_pool(name="pqps", bufs=1, space="PSUM"))
            stps = ctx.enter_context(
                tc.tile_pool(name="stps", bufs=2, space="PSUM"))
            atps = ctx.enter_context(
                tc.tile_pool(name="atps", bufs=1, space="PSUM"))

            def rope_proj(m, xts, w1_sb, w2_sb, dst):
                sl = slice(512 * m, 512 * (m + 1))
                if True:
                    pq = pqps.tile([128, 2, 512], F32, tag="pq", name="pq")
                    for half, w_sb in ((0, w1_sb), (1, w2_sb)):
                        for ko in range(KO):
                            nc.tensor.matmul(pq[:, half, :], w_sb[:, ko],
                                             xts[:, ko],
                                             start=(ko == 0), stop=(ko == KO - 1))
                    # rope: t12 = [x1*cos | x2*sin], t43 = [x1*sin | x2*cos]
                    t12 = work.tile([128, 2, 512], BF16, tag="t12", name="t12")
                    t43 = work.tile([128, 2, 512], BF16, tag="t43", name="t43")
                    nc.vector.tensor_mul(t12[:], pq[:], ccss_sb[:, :, sl])
                    nc.vector.tensor_mul(t43[:], pq[:], sscc_sb[:, :, sl])
                    for h in range(HPC):
                        pr, a = h // 2, h % 2
                        hs = slice(32 * h, 32 * h + 32)
                        nc.vector.tensor_sub(
                            dst[pr][m][64 * a:64 * a + 32, :],
                            t12[hs, 0], t12[hs, 1])
                        nc.vector.tensor_add(
                            dst[pr][m][64 * a + 32:64 * a + 64, :],
                            t43[hs, 1], t43[hs, 0])

            def proj_q(m):
                """x slice DMA + Q projection + RoPE for slice m."""
                xts = work.tile([128, KO, 512], BF16, tag="xts", name="xts")
                xTr = xT.rearrange("(ko p) s -> p ko s", p=128)
                sl = slice(512 * m, 512 * (m + 1))
                nc.sync.dma_start(xts[:, 0:2], xTr[:, 0:2, sl])
                nc.sync.dma_start(xts[:, 2:5], xTr[:, 2:5, sl])
                nc.sync.dma_start(xts[:, 5:8], xTr[:, 5:8, sl])
                rope_proj(m, xts, wq1_sb, wq2_sb, qr)
                return xts

            def proj_kv(m, xts):
                """K projection + RoPE, V projection for slice m."""
                rope_proj(m, xts, wk1_sb, wk2_sb, kr)
                for scp in range(2):
                    pv = pqps.tile([128, 2, 512], F32, tag="pq", name="pv")
                    for sc2 in range(2):
                        sc = 2 * scp + sc2
                        for ko in range(KO):
                            nc.tensor.matmul(
                                pv[:, sc2, 0:DG],
                                xts[:, ko, 128 * sc:128 * sc + 128],
                                wv_sb[:, ko],
                                start=(ko == 0), stop=(ko == KO - 1))
                    nc.scalar.copy(
                        vt[m][:, 2 * scp:2 * scp + 2, :, 0:DH],
                        pv[:, :, 0:DG].rearrange("p s (h d) -> p s h d", d=DH))

            def score_unit(m, p, kb):
                """Emit scores + mask inject + exp for (m, p, kb); return the
                bf16 probs tile and the slice written."""
                km, j = kb // 4, kb % 4
                ksl = slice(128 * j, 128 * j + 128)
                diag = (km == m)
                c0 = 128 * j if diag else 0
                st = stps.tile([128, 2, 512], F32, tag="st", name="st")
                for a in range(2):
                    nc.tensor.matmul(
                        st[:, a, c0:],
                        kr[p][km][64 * a:64 * a + 64, ksl],
                        qr[p][m][64 * a:64 * a + 64, c0:],
                        start=True, stop=not diag)
                if diag:
                    for a in range(2):
                        nc.tensor.matmul(
                            st[:, a, c0:c0 + 128], id_sb[:], mc_sb[:],
                            start=False, stop=True,
                            skip_group_check=True)
                pt = work.tile([128, 2, 512], BF16, tag="pt",
                               name="pt", bufs=8)
                nc.scalar.activation(
                    pt[:, :, c0:], st[:, :, c0:], Exp, scale=EXP_SCALE)
                return pt, c0

            def pv_unit(m, p, kb, n_kb, at, pt, c0):
                km, j = kb // 4, kb % 4
                for a in range(2):
                    nc.tensor.matmul(
                        at[:, a, c0:], vt[km][:, j, 2 * p + a],
                        pt[:, a, c0:],
                        start=(kb == 0), stop=(kb == n_kb - 1))

            def attention(m):
                # software-pipelined across both p-chains: scores/exp run one
                # unit ahead of PV so the PE never head-of-line blocks on exp
                n_kb = 4 * m + 4
                units = [(p, kb) for p in range(2) for kb in range(n_kb)]
                ats = {}
                pending = []
                normalized = []

                def do_normalize(p, at):
                    normalize(m, p, at, last=(m == 3 and p == 1))
                    normalized.append(p)

                for i, (p, kb) in enumerate(units):
                    if kb == 0:
                        ats[p] = atps.tile([DH + 1, 2, 512], F32, tag="at",
                                           name="at")
                    pt, c0 = score_unit(m, p, kb)
                    pending.append((p, kb, pt, c0))
                    if len(pending) > 5:
                        pp, pkb, ppt, pc0 = pending.pop(0)
                        pv_unit(m, pp, pkb, n_kb, ats[pp], ppt, pc0)
                        if pkb == n_kb - 1:
                            do_normalize(pp, ats[pp])
                while pending:
                    pp, pkb, ppt, pc0 = pending.pop(0)
                    pv_unit(m, pp, pkb, n_kb, ats[pp], ppt, pc0)
                    if pkb == n_kb - 1:
                        do_normalize(pp, ats[pp])
            def normalize(m, p, at, last=False):
                # atn = at[0:64] * recip(denominator row)
                if USE_GPSIMD_BCAST and not last:
                    dsum = work.tile([1, 2, 512], F32, tag="dsum", name="dsum")
                    nc.vector.tensor_copy(dsum[:], at[DH:DH + 1])
                    dbc = work.tile([64, 2, 512], F32, tag="dbc", name="dbc")
                    nc.gpsimd.partition_broadcast(dbc[:], dsum[:])
                    rbc = work.tile([64, 2, 512], F32, tag="rbc", name="rbc")
                    nc.vector.reciprocal_approx_fast(rbc[:], dbc[:])
                    for a in range(2):
                        nc.vector.tensor_mul(
                            atn[p][m][64 * a:64 * a + 64, :],
                            at[0:DH, a], rbc[:, a])
                else:
                    F32R = mybir.dt.float32r
                    for a in range(2):
                        ssum = work.tile([1, 512], F32R, tag="ssum",
                                         name="ssum")
                        nc.vector.tensor_copy(ssum[:], at[DH:DH + 1, a])
                        sbc = stps.tile([64, 512], F32, tag="st",
                                        name="sbc")
                        nc.tensor.matmul(sbc[:], ones1[:], ssum[:],
                                         start=True, stop=True)
                        rbc = work.tile([64, 512], F32, tag="rbc",
                                        name="rbc")
                        nc.vector.reciprocal_approx_fast(rbc[:], sbc[:])
                        nc.vector.tensor_mul(
                            atn[p][m][64 * a:64 * a + 64, :],
                            at[0:DH, a], rbc[:])

            def outproj(m):
                for sc in range(4 * m, 4 * m + 4):
                    scl = slice(128 * (sc % 4), 128 * (sc % 4) + 128)
                    if m == 3 and sc % 2 == 1:
                        # tail: attention is drained, st slots are free --
                        # ping-pong po between pools so osb copies overlap MMs
                        po = stps.tile([128, 2, 512], F32, tag="st", name="po")
                    else:
                        po = pqps.tile([128, 2, 512], F32, tag="pq", name="po")
                    for ks in range(2):
                        for nh in range(2):
                            nc.tensor.matmul(
                                po[:, nh, :], atn[ks][sc // 4][:, scl],
                                wo_sb[:, ks, 512 * nh:512 * nh + 512],
                                start=(ks == 0), stop=(ks == 1))
                    osb = work.tile([128, 1024], BF16, tag="osb", name="osb")
                    nc.scalar.copy(
                        osb[:].rearrange("p (x n) -> p x n", x=2), po[:])
                    nc.sync.dma_start(o_part[128 * sc:128 * sc + 128, :], osb[:])

            xts0 = proj_q(0)
            proj_kv(0, xts0)
            xts_next = proj_q(1)
            attention(0)
            proj_kv(1, xts_next)
            xts_next = proj_q(2)
            outproj(0)
            attention(1)
            proj_kv(2, xts_next)
            xts_next = proj_q(3)
            outproj(1)
            attention(2)
            proj_kv(3, xts_next)
            outproj(2)
            attention(3)
            outproj(3)

    nc.finalize()
    return nc


def prep_inputs(hidden_states, cos, sin, attention_mask, Wq, Wk, Wv, Wo):
    """Host-side sharding/layout prep. Returns in_maps for the 8 cores."""
    bf = ml_dtypes.bfloat16
    hs = np.asarray(hidden_states, dtype=np.float32)
    cos = np.asarray(cos, dtype=np.float32)
    sin = np.asarray(sin, dtype=np.float32)
    Wq = np.asarray(Wq, dtype=np.float32)
    Wk = np.asarray(Wk, dtype=np.float32)
    Wv = np.asarray(Wv, dtype=np.float32)
    Wo = np.asarray(Wo, dtype=np.float32)

    # ccss[p, 0, s] = cos[s, p%32]; ccss[p, 1, s] = sin[s, p%32]
    ct = np.tile(cos.T, (4, 1))  # [128, S]
    st_ = np.tile(sin.T, (4, 1))
    ccss = np.ascontiguousarray(np.stack([ct, st_], axis=1).astype(bf))
    sscc = np.ascontiguousarray(np.stack([st_, ct], axis=1).astype(bf))

    idm = np.eye(128, dtype=bf)
    kappa = np.arange(128)[:, None]
    u = np.arange(128)[None, :]
    maskc = np.where(u >= kappa, 0.0, MASK_VAL).astype(bf)

    xTs = [np.ascontiguousarray(hs[b].T.astype(bf)) for b in range(B)]

    in_maps = []
    for c in range(N_CORES):
        b, g = c // 4, c % 4
        hsl = slice(DG * g, DG * (g + 1))
        wq_g = Wq[:, hsl].reshape(D, HPC, DH)
        wk_g = Wk[:, hsl].reshape(D, HPC, DH)
        in_maps.append({
            "xT": xTs[b],
            "wq1": np.ascontiguousarray(
                wq_g[:, :, :32].reshape(D, 128).astype(bf)),
            "wq2": np.ascontiguousarray(
                wq_g[:, :, 32:].reshape(D, 128).astype(bf)),
            "wk1": np.ascontiguousarray(
                wk_g[:, :, :32].reshape(D, 128).astype(bf)),
            "wk2": np.ascontiguousarray(
                wk_g[:, :, 32:].reshape(D, 128).astype(bf)),
            "wv": np.ascontiguousarray(Wv[:, hsl].astype(bf)),
            "wo": np.ascontiguousarray(Wo[hsl, :].astype(bf)),
            "ccss": ccss,
            "sscc": sscc,
            "ident": idm,
            "maskc": maskc,
        })
    return in_maps


_NC_CACHE = {}


def get_nc():
    if "nc" not in _NC_CACHE:
        _NC_CACHE["nc"] = build_nc()
    return _NC_CACHE["nc"]


def run(inputs, trace=False):
    """Returns (output [B,S,D] fp32, BassKernelResults)."""
    nc = get_nc()
    in_maps = prep_inputs(**inputs)
    res = run_bass_kernel_spmd(nc, in_maps, list(range(N_CORES)), trace=trace)
    out = np.zeros((B, S, D), dtype=np.float32)
    for c in range(N_CORES):
        out[c // 4] += res.results[c]["o_part"].astype(np.float32)
    return out, res


def kernel(**inputs):
    return run(inputs, trace=False)[0]


# revision 33
# speedup vs baseline: 1.0562x; 1.0000x over previous
"""Trainium2 Bass kernel for nn_MultiHeadAttention_65987877535893.

MHA with RoPE: B=2, S=2048, D=1024, H=16, Dh=64, causal mask.

Sharding (8 cores): data-parallel over B (x2) x tensor-parallel over heads
(x4 -> 4 heads/core).  Each core computes, for its batch b and head group g:
  QKV projections (column-sharded W), RoPE, causal attention, and a partial
  output  A_g @ Wo_g  (row-sharded Wo).  Host sums the 4 partials per batch.

Final design (all matmul inputs bf16, fp32 PSUM accumulation):
  - projections: lhsT=W chunks (bf16, FWL), rhs = xT slices; Q/K into a
    combined [128,2,512] PSUM tile (x1 dims half 0, x2 dims half 1).
  - RoPE on DVE: 2 fused muls against [cc|ss] / [ss|cc] tables, then 8
    strided sub/add producing head-contiguous bf16 qr/kr tiles.
  - scores TRANSPOSED (ST[k,q] = K @ Q^T), two heads concurrently via PE
    row-tiling (K=64 each, base partitions 0/64); causal mask handled by
    (a) skipping out-of-range blocks, (b) accumulating a constant -3200
    triangular tile into the 128-wide diagonal boundary via an extra
    identity-lhsT matmul (so exp gives exact zeros) -- no DVE mask pass.
  - exp on ScalarE (scale=1/8 folded), bf16 probs.
  - PV: lhsT=[V_h | ones] [128,65] bf16 -> A^T[64,q] + softmax denominator.
  - normalize: reciprocal on DVE, partition-broadcast on GpSimd (attn
    ucode library), two DVE muls -> bf16 atn.
  - output: O = atn^T @ Wo in PSUM, copied to SBUF bf16 on ScalarE,
    DMA'd out as bf16 partials; host sums in fp32.
  PSUM budget: st(tag x2 bufs = 4 banks) + at(2) + pq(2) = 8 banks.
  Scheduling: attention is emitted software-pipelined (scores/exp run 3
  units ahead of PV) across both head-pair chains; Q-proj of slice m+1 is
  hoisted before attention(m) so chain boundaries never wait on RoPE; the
  tail out-projection ping-pongs PSUM slots with the drained score pool.
  Engine balance: PE ~matmuls, ScalarE = exp + PSUM->SBUF copies,
  VectorE = RoPE + normalize muls, GpSimd = denominator broadcast.
"""

import os
import sys

sys.path.insert(0, "/opt/trn_rl_repo")
os.environ.setdefault("MYCRO_LOCAL_CACHE", "1")

import numpy as np
import ml_dtypes

import concourse.bass as bass
import concourse.bacc as bacc
import concourse.mybir as mybir
import concourse.tile as tile
from concourse import library_config
from concourse.bass_utils import run_bass_kernel_spmd

F32 = mybir.dt.float32
BF16 = mybir.dt.bfloat16

B, S, D = 2, 2048, 1024
H = 16
DH = 64
HPC = 4           # heads per core
DG = HPC * DH     # 256
N_CORES = 8
KO = D // 128     # 8 contraction chunks
N_SLICES = S // 512   # 4 q/s slices
EXP_SCALE = float(DH) ** -0.5  # 0.125
MASK_VAL = -3200.0
Exp = mybir.ActivationFunctionType.Exp

USE_GPSIMD_BCAST = True


def build_nc():
    nc = bacc.Bacc()

    xT = nc.dram_tensor("xT", [D, S], BF16, kind="ExternalInput")
    wq1 = nc.dram_tensor("wq1", [D, 128], BF16, kind="ExternalInput")
    wq2 = nc.dram_tensor("wq2", [D, 128], BF16, kind="ExternalInput")
    wk1 = nc.dram_tensor("wk1", [D, 128], BF16, kind="ExternalInput")
    wk2 = nc.dram_tensor("wk2", [D, 128], BF16, kind="ExternalInput")
    wv = nc.dram_tensor("wv", [D, DG], BF16, kind="ExternalInput")
    wo = nc.dram_tensor("wo", [DG, D], BF16, kind="ExternalInput")
    ccss = nc.dram_tensor("ccss", [128, 2, S], BF16, kind="ExternalInput")
    sscc = nc.dram_tensor("sscc", [128, 2, S], BF16, kind="ExternalInput")
    ident = nc.dram_tensor("ident", [128, 128], BF16, kind="ExternalInput")
    maskc = nc.dram_tensor("maskc", [128, 128], BF16, kind="ExternalInput")
    o_part = nc.dram_tensor("o_part", [S, D], BF16, kind="ExternalOutput")

    with tile.TileContext(nc) as tc:
        import contextlib

        ctx = contextlib.ExitStack()
        with ctx:
            if USE_GPSIMD_BCAST:
                nc.gpsimd.load_library(library_config.attn)

            persist = ctx.enter_context(tc.tile_pool(name="persist", bufs=1))
            work = ctx.enter_context(tc.tile_pool(name="work", bufs=2))

            # ---- persistent SBUF tensors ----
            qr = [[persist.tile([128, 512], BF16, tag=f"qr{p}_{ms}", name=f"qr{p}_{ms}")
                   for ms in range(N_SLICES)] for p in range(2)]
            kr = [[persist.tile([128, 512], BF16, tag=f"kr{p}_{ms}", name=f"kr{p}_{ms}")
                   for ms in range(N_SLICES)] for p in range(2)]
            # V with a ones column per (kb, head): [128, kb_in_slice, head, 65]
            vt = [persist.tile([128, 4, HPC, DH + 1], BF16, tag=f"vt{ms}", name=f"vt{ms}")
                  for ms in range(N_SLICES)]
            atn = [[persist.tile([128, 512], BF16, tag=f"atn{p}_{ms}", name=f"atn{p}_{ms}")
                    for ms in range(N_SLICES)] for p in range(2)]
            ccss_sb = persist.tile([128, 2, S], BF16, tag="ccss", name="ccss_sb")
            sscc_sb = persist.tile([128, 2, S], BF16, tag="sscc", name="sscc_sb")
            id_sb = persist.tile([128, 128], BF16, tag="id", name="id_sb")
            mc_sb = persist.tile([128, 128], BF16, tag="mc", name="mc_sb")
            wq1_sb = persist.tile([128, KO, 128], BF16, tag="wq1", name="wq1_sb")
            wq2_sb = persist.tile([128, KO, 128], BF16, tag="wq2", name="wq2_sb")
            wk1_sb = persist.tile([128, KO, 128], BF16, tag="wk1", name="wk1_sb")
            wk2_sb = persist.tile([128, KO, 128], BF16, tag="wk2", name="wk2_sb")
            wv_sb = persist.tile([128, KO, DG], BF16, tag="wv", name="wv_sb")
            wo_sb = persist.tile([128, 2, D], BF16, tag="wo", name="wo_sb")

            # preload order tuned for the m=0 critical path: Q weights,
            # slice-0 rope tables, K weights, V weights, then the rest
            wq1r = wq1.rearrange("(ko p) m -> p ko m", p=128)
            nc.scalar.dma_start(wq1_sb[:, 0:2], wq1r[:, 0:2])
            nc.scalar.dma_start(wq1_sb[:, 2:8], wq1r[:, 2:8])
            nc.scalar.dma_start(
                wq2_sb[:], wq2.rearrange("(ko p) m -> p ko m", p=128))
            nc.scalar.dma_start(ccss_sb[:, :, 0:512], ccss[:, :, 0:512])
            nc.gpsimd.dma_start(sscc_sb[:, :, 0:512], sscc[:, :, 0:512])
            nc.gpsimd.dma_start(
                wk1_sb[:], wk1.rearrange("(ko p) m -> p ko m", p=128))
            nc.gpsimd.dma_start(
                wk2_sb[:], wk2.rearrange("(ko p) m -> p ko m", p=128))
            nc.scalar.dma_start(
                wv_sb[:], wv.rearrange("(ko p) m -> p ko m", p=128))
            nc.sync.dma_start(id_sb[:], ident[:])
            nc.sync.dma_start(mc_sb[:], maskc[:])
            for ms in range(1, N_SLICES):
                sl = slice(512 * ms, 512 * (ms + 1))
                nc.scalar.dma_start(ccss_sb[:, :, sl], ccss[:, :, sl])
                nc.gpsimd.dma_start(sscc_sb[:, :, sl], sscc[:, :, sl])
            nc.scalar.dma_start(
                wo_sb[:], wo.rearrange("(ko p) m -> p ko m", p=128))

            # ones column of vt
            for ms in range(N_SLICES):
                nc.vector.memset(vt[ms][:, :, :, DH], 1.0)

            if True:
                F32R = mybir.dt.float32r
                onesf = persist.tile([128, 64], F32, tag="onesf", name="onesf")
                ones1 = persist.tile([1, 64], F32R, tag="ones1", name="ones1")
                nc.vector.memset(onesf[:], 1.0)
                nc.vector.tensor_copy(ones1[:], onesf[0:1, :])

            # ---- PSUM pools ----
            pqps = ctx.enter_context(
                tc.tile_pool(name="pqps", bufs=1, space="PSUM"))
            stps = ctx.enter_context(
                tc.tile_pool(name="stps", bufs=2, space="PSUM"))
            atps = ctx.enter_context(
                tc.tile_pool(name="atps", bufs=1, space="PSUM"))

            def rope_proj(m, xts, w1_sb, w2_sb, dst):
                sl = slice(512 * m, 512 * (m + 1))
                if True:
                    pq = pqps.tile([128, 2, 512], F32, tag="pq", name="pq")
                    for half, w_sb in ((0, w1_sb), (1, w2_sb)):
                        for ko in range(KO):
                            nc.tensor.matmul(pq[:, half, :], w_sb[:, ko],
                                             xts[:, ko],
                                             start=(ko == 0), stop=(ko == KO - 1))
                    # rope: t12 = [x1*cos | x2*sin], t43 = [x1*sin | x2*cos]
                    t12 = work.tile([128, 2, 512], BF16, tag="t12", name="t12")
                    t43 = work.tile([128, 2, 512], BF16, tag="t43", name="t43")
                    nc.vector.tensor_mul(t12[:], pq[:], ccss_sb[:, :, sl])
                    nc.vector.tensor_mul(t43[:], pq[:], sscc_sb[:, :, sl])
                    for h in range(HPC):
                        pr, a = h // 2, h % 2
                        hs = slice(32 * h, 32 * h + 32)
                        nc.vector.tensor_sub(
                            dst[pr][m][64 * a:64 * a + 32, :],
                            t12[hs, 0], t12[hs, 1])
                        nc.vector.tensor_add(
                            dst[pr][m][64 * a + 32:64 * a + 64, :],
                            t43[hs, 1], t43[hs, 0])

            def proj_q(m):
                """x slice DMA + Q projection + RoPE for slice m."""
                xts = work.tile([128, KO, 512], BF16, tag="xts", name="xts")
                xTr = xT.rearrange("(ko p) s -> p ko s", p=128)
                sl = slice(512 * m, 512 * (m + 1))
                nc.sync.dma_start(xts[:, 0:2], xTr[:, 0:2, sl])
                nc.sync.dma_start(xts[:, 2:5], xTr[:, 2:5, sl])
                nc.sync.dma_start(xts[:, 5:8], xTr[:, 5:8, sl])
                rope_proj(m, xts, wq1_sb, wq2_sb, qr)
                return xts

            def proj_kv(m, xts):
                """K projection + RoPE, V projection for slice m."""
                rope_proj(m, xts, wk1_sb, wk2_sb, kr)
                for scp in range(2):
                    pv = pqps.tile([128, 2, 512], F32, tag="pq", name="pv")
                    for sc2 in range(2):
                        sc = 2 * scp + sc2
                        for ko in range(KO):
                            nc.tensor.matmul(
                                pv[:, sc2, 0:DG],
                                xts[:, ko, 128 * sc:128 * sc + 128],
                                wv_sb[:, ko],
                                start=(ko == 0), stop=(ko == KO - 1))
                    nc.scalar.copy(
                        vt[m][:, 2 * scp:2 * scp + 2, :, 0:DH],
                        pv[:, :, 0:DG].rearrange("p s (h d) -> p s h d", d=DH))

            def score_unit(m, p, kb):
                """Emit scores + mask inject + exp for (m, p, kb); return the
                bf16 probs tile and the slice written."""
                km, j = kb // 4, kb % 4
                ksl = slice(128 * j, 128 * j + 128)
                diag = (km == m)
                c0 = 128 * j if diag else 0
                st = stps.tile([128, 2, 512], F32, tag="st", name="st")
                for a in range(2):
                    nc.tensor.matmul(
                        st[:, a, c0:],
                        kr[p][km][64 * a:64 * a + 64, ksl],
                        qr[p][m][64 * a:64 * a + 64, c0:],
                        start=True, stop=not diag)
                if diag:
                    for a in range(2):
                        nc.tensor.matmul(
                            st[:, a, c0:c0 + 128], id_sb[:], mc_sb[:],
                            start=False, stop=True,
                            skip_group_check=True)
                pt = work.tile([128, 2, 512], BF16, tag="pt",
                               name="pt", bufs=10)
                nc.scalar.activation(
                    pt[:, :, c0:], st[:, :, c0:], Exp, scale=EXP_SCALE)
                return pt, c0

            def pv_unit(m, p, kb, n_kb, at, pt, c0):
                km, j = kb // 4, kb % 4
                for a in range(2):
                    nc.tensor.matmul(
                        at[:, a, c0:], vt[km][:, j, 2 * p + a],
                        pt[:, a, c0:],
                        start=(kb == 0), stop=(kb == n_kb - 1))

            def attention(m):
                # software-pipelined across both p-chains: scores/exp run one
                # unit ahead of PV so the PE never head-of-line blocks on exp
                n_kb = 4 * m + 4
                units = [(p, kb) for p in range(2) for kb in range(n_kb)]
                ats = {}
                pending = []
                normalized = []

                def do_normalize(p, at):
                    normalize(m, p, at, last=(m == 3 and p == 1))
                    normalized.append(p)

                for i, (p, kb) in enumerate(units):
                    if kb == 0:
                        ats[p] = atps.tile([DH + 1, 2, 512], F32, tag="at",
                                           name="at")
                    pt, c0 = score_unit(m, p, kb)
                    pending.append((p, kb, pt, c0))
                    if len(pending) > 7:
                        pp, pkb, ppt, pc0 = pending.pop(0)
                        pv_unit(m, pp, pkb, n_kb, ats[pp], ppt, pc0)
                        if pkb == n_kb - 1:
                            do_normalize(pp, ats[pp])
                while pending:
                    pp, pkb, ppt, pc0 = pending.pop(0)
                    pv_unit(m, pp, pkb, n_kb, ats[pp], ppt, pc0)
                    if pkb == n_kb - 1:
                        do_normalize(pp, ats[pp])
            def normalize(m, p, at, last=False):
                # atn = at[0:64] * recip(denominator row)
                if USE_GPSIMD_BCAST and not last:
                    dsum = work.tile([1, 2, 512], F32, tag="dsum", name="dsum")
                    nc.vector.tensor_copy(dsum[:], at[DH:DH + 1])
                    dbc = work.tile([64, 2, 512], F32, tag="dbc", name="dbc")
                    nc.gpsimd.partition_broadcast(dbc[:], dsum[:])
                    rbc = work.tile([64, 2, 512], F32, tag="rbc", name="rbc")
                    nc.vector.reciprocal_approx_fast(rbc[:], dbc[:])
                    for a in range(2):
                        nc.vector.tensor_mul(
                            atn[p][m][64 * a:64 * a + 64, :],
                            at[0:DH, a], rbc[:, a])
                else:
                    F32R = mybir.dt.float32r
                    for a in range(2):
                        ssum = work.tile([1, 512], F32R, tag="ssum",
                                         name="ssum")
                        nc.vector.tensor_copy(ssum[:], at[DH:DH + 1, a])
                        sbc = stps.tile([64, 512], F32, tag="st",
                                        name="sbc")
                        nc.tensor.matmul(sbc[:], ones1[:], ssum[:],
                                         start=True, stop=True)
                        rbc = work.tile([64, 512], F32, tag="rbc",
                                        name="rbc")
                        nc.vector.reciprocal_approx_fast(rbc[:], sbc[:])
                        nc.vector.tensor_mul(
                            atn[p][m][64 * a:64 * a + 64, :],
                            at[0:DH, a], rbc[:])

            def outproj(m):
                for sc in range(4 * m, 4 * m + 4):
                    scl = slice(128 * (sc % 4), 128 * (sc % 4) + 128)
                    if m == 3 and sc % 2 == 1:
                        # tail: attention is drained, st slots are free --
                        # ping-pong po between pools so osb copies overlap MMs
                        po = stps.tile([128, 2, 512], F32, tag="st", name="po")
                    else:
                        po = pqps.tile([128, 2, 512], F32, tag="pq", name="po")
                    for ks in range(2):
                        for nh in range(2):
                            nc.tensor.matmul(
                                po[:, nh, :], atn[ks][sc // 4][:, scl],
                                wo_sb[:, ks, 512 * nh:512 * nh + 512],
                                start=(ks == 0), stop=(ks == 1))
                    osb = work.tile([128, 1024], BF16, tag="osb", name="osb")
                    nc.scalar.copy(
                        osb[:].rearrange("p (x n) -> p x n", x=2), po[:])
                    nc.sync.dma_start(o_part[128 * sc:128 * sc + 128, :], osb[:])

            xts0 = proj_q(0)
            proj_kv(0, xts0)
            xts_next = proj_q(1)
            attention(0)
            proj_kv(1, xts_next)
            xts_next = proj_q(2)
            outproj(0)
            attention(1)
            proj_kv(2, xts_next)
            xts_next = proj_q(3)
            outproj(1)
            attention(2)
            proj_kv(3, xts_next)
            outproj(2)
            attention(3)
            outproj(3)

    nc.finalize()
    return nc


def prep_inputs(hidden_states, cos, sin, attention_mask, Wq, Wk, Wv, Wo):
    """Host-side sharding/layout prep. Returns in_maps for the 8 cores."""
    bf = ml_dtypes.bfloat16
    hs = np.asarray(hidden_states, dtype=np.float32)
    cos = np.asarray(cos, dtype=np.float32)
    sin = np.asarray(sin, dtype=np.float32)
    Wq = np.asarray(Wq, dtype=np.float32)
    Wk = np.asarray(Wk, dtype=np.float32)
    Wv = np.asarray(Wv, dtype=np.float32)
    Wo = np.asarray(Wo, dtype=np.float32)

    # ccss[p, 0, s] = cos[s, p%32]; ccss[p, 1, s] = sin[s, p%32]
    ct = np.tile(cos.T, (4, 1))  # [128, S]
    st_ = np.tile(sin.T, (4, 1))
    ccss = np.ascontiguousarray(np.stack([ct, st_], axis=1).astype(bf))
    sscc = np.ascontiguousarray(np.stack([st_, ct], axis=1).astype(bf))

    idm = np.eye(128, dtype=bf)
    kappa = np.arange(128)[:, None]
    u = np.arange(128)[None, :]
    maskc = np.where(u >= kappa, 0.0, MASK_VAL).astype(bf)

    xTs = [np.ascontiguousarray(hs[b].T.astype(bf)) for b in range(B)]

    in_maps = []
    for c in range(N_CORES):
        b, g = c // 4, c % 4
        hsl = slice(DG * g, DG * (g + 1))
        wq_g = Wq[:, hsl].reshape(D, HPC, DH)
        wk_g = Wk[:, hsl].reshape(D, HPC, DH)
        in_maps.append({
            "xT": xTs[b],
            "wq1": np.ascontiguousarray(
                wq_g[:, :, :32].reshape(D, 128).astype(bf)),
            "wq2": np.ascontiguousarray(
                wq_g[:, :, 32:].reshape(D, 128).astype(bf)),
            "wk1": np.ascontiguousarray(
                wk_g[:, :, :32].reshape(D, 128).astype(bf)),
            "wk2": np.ascontiguousarray(
                wk_g[:, :, 32:].reshape(D, 128).astype(bf)),
            "wv": np.ascontiguousarray(Wv[:, hsl].astype(bf)),
            "wo": np.ascontiguousarray(Wo[hsl, :].astype(bf)),
            "ccss": ccss,
            "sscc": sscc,
            "ident": idm,
            "maskc": maskc,
        })
    return in_maps


_NC_CACHE = {}


def get_nc():
    if "nc" not in _NC_CACHE:
        _NC_CACHE["nc"] = build_nc()
    return _NC_CACHE["nc"]


def run(inputs, trace=False):
    """Returns (output [B,S,D] fp32, BassKernelResults)."""
    nc = get_nc()
    in_maps = prep_inputs(**inputs)
    res = run_bass_kernel_spmd(nc, in_maps, list(range(N_CORES)), trace=trace)
    out = np.zeros((B, S, D), dtype=np.float32)
    for c in range(N_CORES):
        out[c // 4] += res.results[c]["o_part"].astype(np.float32)
    return out, res


def kernel(**inputs):
    return run(inputs, trace=False)[0]


# revision 34
# speedup vs baseline: 1.0762x; 1.0190x over previous
"""Trainium2 Bass kernel for nn_MultiHeadAttention_65987877535893.

MHA with RoPE: B=2, S=2048, D=1024, H=16, Dh=64, causal mask.

Sharding (8 cores): data-parallel over B (x2) x tensor-parallel over heads
(x4 -> 4 heads/core).  Each core computes, for its batch b and head group g:
  QKV projections (column-sharded W), RoPE, causal attention, and a partial
  output  A_g @ Wo_g  (row-sharded Wo).  Host sums the 4 partials per batch.

Final design (all matmul inputs bf16, fp32 PSUM accumulation):
  - projections: lhsT=W chunks (bf16, FWL), rhs = xT slices; Q/K into a
    combined [128,2,512] PSUM tile (x1 dims half 0, x2 dims half 1).
  - RoPE on DVE: 2 fused muls against [cc|ss] / [ss|cc] tables, then 8
    strided sub/add producing head-contiguous bf16 qr/kr tiles.
  - scores TRANSPOSED (ST[k,q] = K @ Q^T), two heads concurrently via PE
    row-tiling (K=64 each, base partitions 0/64); causal mask handled by
    (a) skipping out-of-range blocks, (b) accumulating a constant -3200
    triangular tile into the 128-wide diagonal boundary via an extra
    identity-lhsT matmul (so exp gives exact zeros) -- no DVE mask pass.
  - exp on ScalarE (scale=1/8 folded), bf16 probs.
  - PV: lhsT=[V_h | ones] [128,65] bf16 -> A^T[64,q] + softmax denominator.
  - normalize: reciprocal on DVE, partition-broadcast on GpSimd (attn
    ucode library), two DVE muls -> bf16 atn.
  - output: O = atn^T @ Wo in PSUM, copied to SBUF bf16 on ScalarE,
    DMA'd out as bf16 partials; host sums in fp32.
  PSUM budget: st(tag x2 bufs = 4 banks) + at(2) + pq(2) = 8 banks.
  Scheduling: attention is emitted software-pipelined (scores/exp run 3
  units ahead of PV) across both head-pair chains; Q-proj of slice m+1 is
  hoisted before attention(m) so chain boundaries never wait on RoPE; the
  tail out-projection ping-pongs PSUM slots with the drained score pool.
  Engine balance: PE ~matmuls, ScalarE = exp + PSUM->SBUF copies,
  VectorE = RoPE + normalize muls, GpSimd = denominator broadcast.
"""

import os
import sys

sys.path.insert(0, "/opt/trn_rl_repo")
os.environ.setdefault("MYCRO_LOCAL_CACHE", "1")

import numpy as np
import ml_dtypes

import concourse.bass as bass
import concourse.bacc as bacc
import concourse.mybir as mybir
import concourse.tile as tile
from concourse import library_config
from concourse.bass_utils import run_bass_kernel_spmd

F32 = mybir.dt.float32
BF16 = mybir.dt.bfloat16

B, S, D = 2, 2048, 1024
H = 16
DH = 64
HPC = 4           # heads per core
DG = HPC * DH     # 256
N_CORES = 8
KO = D // 128     # 8 contraction chunks
N_SLICES = S // 512   # 4 q/s slices
EXP_SCALE = float(DH) ** -0.5  # 0.125
MASK_VAL = -3200.0
Exp = mybir.ActivationFunctionType.Exp

USE_GPSIMD_BCAST = True


def build_nc():
    nc = bacc.Bacc()

    xT = nc.dram_tensor("xT", [D, S], BF16, kind="ExternalInput")
    wq1 = nc.dram_tensor("wq1", [D, 128], BF16, kind="ExternalInput")
    wq2 = nc.dram_tensor("wq2", [D, 128], BF16, kind="ExternalInput")
    wk1 = nc.dram_tensor("wk1", [D, 128], BF16, kind="ExternalInput")
    wk2 = nc.dram_tensor("wk2", [D, 128], BF16, kind="ExternalInput")
    wv = nc.dram_tensor("wv", [D, DG], BF16, kind="ExternalInput")
    wo = nc.dram_tensor("wo", [DG, D], BF16, kind="ExternalInput")
    ccss = nc.dram_tensor("ccss", [128, 2, S], BF16, kind="ExternalInput")
    sscc = nc.dram_tensor("sscc", [128, 2, S], BF16, kind="ExternalInput")
    ident = nc.dram_tensor("ident", [128, 128], BF16, kind="ExternalInput")
    maskc = nc.dram_tensor("maskc", [128, 128], BF16, kind="ExternalInput")
    o_part = nc.dram_tensor("o_part", [S, D], BF16, kind="ExternalOutput")

    with tile.TileContext(nc) as tc:
        import contextlib

        ctx = contextlib.ExitStack()
        with ctx:
            if USE_GPSIMD_BCAST:
                nc.gpsimd.load_library(library_config.attn)

            persist = ctx.enter_context(tc.tile_pool(name="persist", bufs=1))
            work = ctx.enter_context(tc.tile_pool(name="work", bufs=3))

            # ---- persistent SBUF tensors ----
            qr = [[persist.tile([128, 512], BF16, tag=f"qr{p}_{ms}", name=f"qr{p}_{ms}")
                   for ms in range(N_SLICES)] for p in range(2)]
            kr = [[persist.tile([128, 512], BF16, tag=f"kr{p}_{ms}", name=f"kr{p}_{ms}")
                   for ms in range(N_SLICES)] for p in range(2)]
            # V with a ones column per (kb, head): [128, kb_in_slice, head, 65]
            vt = [persist.tile([128, 4, HPC, DH + 1], BF16, tag=f"vt{ms}", name=f"vt{ms}")
                  for ms in range(N_SLICES)]
            atn = [[persist.tile([128, 512], BF16, tag=f"atn{p}_{ms}", name=f"atn{p}_{ms}")
                    for ms in range(N_SLICES)] for p in range(2)]
            ccss_sb = persist.tile([128, 2, S], BF16, tag="ccss", name="ccss_sb")
            sscc_sb = persist.tile([128, 2, S], BF16, tag="sscc", name="sscc_sb")
            id_sb = persist.tile([128, 128], BF16, tag="id", name="id_sb")
            mc_sb = persist.tile([128, 128], BF16, tag="mc", name="mc_sb")
            wq1_sb = persist.tile([128, KO, 128], BF16, tag="wq1", name="wq1_sb")
            wq2_sb = persist.tile([128, KO, 128], BF16, tag="wq2", name="wq2_sb")
            wk1_sb = persist.tile([128, KO, 128], BF16, tag="wk1", name="wk1_sb")
            wk2_sb = persist.tile([128, KO, 128], BF16, tag="wk2", name="wk2_sb")
            wv_sb = persist.tile([128, KO, DG], BF16, tag="wv", name="wv_sb")
            wo_sb = persist.tile([128, 2, D], BF16, tag="wo", name="wo_sb")

            # preload order tuned for the m=0 critical path: Q weights,
            # slice-0 rope tables, K weights, V weights, then the rest
            wq1r = wq1.rearrange("(ko p) m -> p ko m", p=128)
            nc.scalar.dma_start(wq1_sb[:, 0:2], wq1r[:, 0:2])
            nc.scalar.dma_start(wq1_sb[:, 2:8], wq1r[:, 2:8])
            nc.scalar.dma_start(
                wq2_sb[:], wq2.rearrange("(ko p) m -> p ko m", p=128))
            nc.scalar.dma_start(ccss_sb[:, :, 0:512], ccss[:, :, 0:512])
            nc.gpsimd.dma_start(sscc_sb[:, :, 0:512], sscc[:, :, 0:512])
            nc.gpsimd.dma_start(
                wk1_sb[:], wk1.rearrange("(ko p) m -> p ko m", p=128))
            nc.gpsimd.dma_start(
                wk2_sb[:], wk2.rearrange("(ko p) m -> p ko m", p=128))
            nc.scalar.dma_start(
                wv_sb[:], wv.rearrange("(ko p) m -> p ko m", p=128))
            nc.sync.dma_start(id_sb[:], ident[:])
            nc.sync.dma_start(mc_sb[:], maskc[:])
            for ms in range(1, N_SLICES):
                sl = slice(512 * ms, 512 * (ms + 1))
                nc.scalar.dma_start(ccss_sb[:, :, sl], ccss[:, :, sl])
                nc.gpsimd.dma_start(sscc_sb[:, :, sl], sscc[:, :, sl])
            nc.scalar.dma_start(
                wo_sb[:], wo.rearrange("(ko p) m -> p ko m", p=128))

            # ones column of vt
            for ms in range(N_SLICES):
                nc.vector.memset(vt[ms][:, :, :, DH], 1.0)

            if True:
                F32R = mybir.dt.float32r
                onesf = persist.tile([128, 64], F32, tag="onesf", name="onesf")
                ones1 = persist.tile([1, 64], F32R, tag="ones1", name="ones1")
                nc.vector.memset(onesf[:], 1.0)
                nc.vector.tensor_copy(ones1[:], onesf[0:1, :])

            # ---- PSUM pools ----
            pqps = ctx.enter_context(
                tc.tile_pool(name="pqps", bufs=1, space="PSUM"))
            stps = ctx.enter_context(
                tc.tile_pool(name="stps", bufs=2, space="PSUM"))
            atps = ctx.enter_context(
                tc.tile_pool(name="atps", bufs=1, space="PSUM"))

            def rope_proj(m, xts, w1_sb, w2_sb, dst):
                sl = slice(512 * m, 512 * (m + 1))
                if True:
                    pq = pqps.tile([128, 2, 512], F32, tag="pq", name="pq")
                    for half, w_sb in ((0, w1_sb), (1, w2_sb)):
                        for ko in range(KO):
                            nc.tensor.matmul(pq[:, half, :], w_sb[:, ko],
                                             xts[:, ko],
                                             start=(ko == 0), stop=(ko == KO - 1))
                    # rope: t12 = [x1*cos | x2*sin], t43 = [x1*sin | x2*cos]
                    t12 = work.tile([128, 2, 512], BF16, tag="t12", name="t12")
                    t43 = work.tile([128, 2, 512], BF16, tag="t43", name="t43")
                    nc.vector.tensor_mul(t12[:], pq[:], ccss_sb[:, :, sl])
                    nc.vector.tensor_mul(t43[:], pq[:], sscc_sb[:, :, sl])
                    for h in range(HPC):
                        pr, a = h // 2, h % 2
                        hs = slice(32 * h, 32 * h + 32)
                        nc.vector.tensor_sub(
                            dst[pr][m][64 * a:64 * a + 32, :],
                            t12[hs, 0], t12[hs, 1])
                        nc.vector.tensor_add(
                            dst[pr][m][64 * a + 32:64 * a + 64, :],
                            t43[hs, 1], t43[hs, 0])

            def proj_q(m):
                """x slice DMA + Q projection + RoPE for slice m."""
                xts = work.tile([128, KO, 512], BF16, tag="xts", name="xts")
                xTr = xT.rearrange("(ko p) s -> p ko s", p=128)
                sl = slice(512 * m, 512 * (m + 1))
                nc.sync.dma_start(xts[:, 0:2], xTr[:, 0:2, sl])
                nc.sync.dma_start(xts[:, 2:5], xTr[:, 2:5, sl])
                nc.sync.dma_start(xts[:, 5:8], xTr[:, 5:8, sl])
                rope_proj(m, xts, wq1_sb, wq2_sb, qr)
                return xts

            def proj_kv(m, xts):
                """K projection + RoPE, V projection for slice m."""
                rope_proj(m, xts, wk1_sb, wk2_sb, kr)
                for scp in range(2):
                    pv = pqps.tile([128, 2, 512], F32, tag="pq", name="pv")
                    for sc2 in range(2):
                        sc = 2 * scp + sc2
                        for ko in range(KO):
                            nc.tensor.matmul(
                                pv[:, sc2, 0:DG],
                                xts[:, ko, 128 * sc:128 * sc + 128],
                                wv_sb[:, ko],
                                start=(ko == 0), stop=(ko == KO - 1))
                    nc.scalar.copy(
                        vt[m][:, 2 * scp:2 * scp + 2, :, 0:DH],
                        pv[:, :, 0:DG].rearrange("p s (h d) -> p s h d", d=DH))

            def score_unit(m, p, kb):
                """Emit scores + mask inject + exp for (m, p, kb); return the
                bf16 probs tile and the slice written."""
                km, j = kb // 4, kb % 4
                ksl = slice(128 * j, 128 * j + 128)
                diag = (km == m)
                c0 = 128 * j if diag else 0
                st = stps.tile([128, 2, 512], F32, tag="st", name="st")
                for a in range(2):
                    nc.tensor.matmul(
                        st[:, a, c0:],
                        kr[p][km][64 * a:64 * a + 64, ksl],
                        qr[p][m][64 * a:64 * a + 64, c0:],
                        start=True, stop=not diag)
                if diag:
                    for a in range(2):
                        nc.tensor.matmul(
                            st[:, a, c0:c0 + 128], id_sb[:], mc_sb[:],
                            start=False, stop=True,
                            skip_group_check=True)
                pt = work.tile([128, 2, 512], BF16, tag="pt",
                               name="pt", bufs=10)
                nc.scalar.activation(
                    pt[:, :, c0:], st[:, :, c0:], Exp, scale=EXP_SCALE)
                return pt, c0

            def pv_unit(m, p, kb, n_kb, at, pt, c0):
                km, j = kb // 4, kb % 4
                for a in range(2):
                    nc.tensor.matmul(
                        at[:, a, c0:], vt[km][:, j, 2 * p + a],
                        pt[:, a, c0:],
                        start=(kb == 0), stop=(kb == n_kb - 1))

            def attention(m):
                # software-pipelined across both p-chains: scores/exp run one
                # unit ahead of PV so the PE never head-of-line blocks on exp
                n_kb = 4 * m + 4
                units = [(p, kb) for p in range(2) for kb in range(n_kb)]
                ats = {}
                pending = []
                normalized = []

                def do_normalize(p, at):
                    normalize(m, p, at, last=(m == 3 and p == 1))
                    normalized.append(p)

                for i, (p, kb) in enumerate(units):
                    if kb == 0:
                        ats[p] = atps.tile([DH + 1, 2, 512], F32, tag="at",
                                           name="at")
                    pt, c0 = score_unit(m, p, kb)
                    pending.append((p, kb, pt, c0))
                    if len(pending) > 7:
                        pp, pkb, ppt, pc0 = pending.pop(0)
                        pv_unit(m, pp, pkb, n_kb, ats[pp], ppt, pc0)
                        if pkb == n_kb - 1:
                            do_normalize(pp, ats[pp])
                while pending:
                    pp, pkb, ppt, pc0 = pending.pop(0)
                    pv_unit(m, pp, pkb, n_kb, ats[pp], ppt, pc0)
                    if pkb == n_kb - 1:
                        do_normalize(pp, ats[pp])
            def normalize(m, p, at, last=False):
                # atn = at[0:64] * recip(denominator row)
                if USE_GPSIMD_BCAST and not last:
                    dsum = work.tile([1, 2, 512], F32, tag="dsum", name="dsum")
                    nc.vector.tensor_copy(dsum[:], at[DH:DH + 1])
                    dbc = work.tile([64, 2, 512], F32, tag="dbc", name="dbc")
                    nc.gpsimd.partition_broadcast(dbc[:], dsum[:])
                    rbc = work.tile([64, 2, 512], F32, tag="rbc", name="rbc")
                    nc.vector.reciprocal_approx_fast(rbc[:], dbc[:])
                    for a in range(2):
                        nc.vector.tensor_mul(
                            atn[p][m][64 * a:64 * a + 64, :],
                            at[0:DH, a], rbc[:, a])
                else:
                    F32R = mybir.dt.float32r
                    for a in range(2):
                        ssum = work.tile([1, 512], F32R, tag="ssum",
                                         name="ssum")
                        nc.vector.tensor_copy(ssum[:], at[DH:DH + 1, a])
                        sbc = stps.tile([64, 512], F32, tag="st",
                                        name="sbc")
                        nc.tensor.matmul(sbc[:], ones1[:], ssum[:],
                                         start=True, stop=True)
                        rbc = work.tile([64, 512], F32, tag="rbc",
                                        name="rbc")
                        nc.vector.reciprocal_approx_fast(rbc[:], sbc[:])
                        nc.vector.tensor_mul(
                            atn[p][m][64 * a:64 * a + 64, :],
                            at[0:DH, a], rbc[:])

            def outproj(m):
                for sc in range(4 * m, 4 * m + 4):
                    scl = slice(128 * (sc % 4), 128 * (sc % 4) + 128)
                    if m == 3 and sc % 2 == 1:
                        # tail: attention is drained, st slots are free --
                        # ping-pong po between pools so osb copies overlap MMs
                        po = stps.tile([128, 2, 512], F32, tag="st", name="po")
                    else:
                        po = pqps.tile([128, 2, 512], F32, tag="pq", name="po")
                    for ks in range(2):
                        for nh in range(2):
                            nc.tensor.matmul(
                                po[:, nh, :], atn[ks][sc // 4][:, scl],
                                wo_sb[:, ks, 512 * nh:512 * nh + 512],
                                start=(ks == 0), stop=(ks == 1))
                    osb = work.tile([128, 1024], BF16, tag="osb", name="osb")
                    nc.scalar.copy(
                        osb[:].rearrange("p (x n) -> p x n", x=2), po[:])
                    nc.sync.dma_start(o_part[128 * sc:128 * sc + 128, :], osb[:])

            xts0 = proj_q(0)
            proj_kv(0, xts0)
            xts_next = proj_q(1)
            attention(0)
            proj_kv(1, xts_next)
            xts_next = proj_q(2)
            outproj(0)
            attention(1)
            proj_kv(2, xts_next)
            xts_next = proj_q(3)
            outproj(1)
            attention(2)
            proj_kv(3, xts_next)
            outproj(2)
            attention(3)
            outproj(3)

    nc.finalize()
    return nc


def prep_inputs(hidden_states, cos, sin, attention_mask, Wq, Wk, Wv, Wo):
    """Host-side sharding/layout prep. Returns in_maps for the 8 cores."""
    bf = ml_dtypes.bfloat16
    hs = np.asarray(hidden_states, dtype=np.float32)
    cos = np.asarray(cos, dtype=np.float32)
    sin = np.asarray(sin, dtype=np.float32)
    Wq = np.asarray(Wq, dtype=np.float32)
    Wk = np.asarray(Wk, dtype=np.float32)
    Wv = np.asarray(Wv, dtype=np.float32)
    Wo = np.asarray(Wo, dtype=np.float32)

    # ccss[p, 0, s] = cos[s, p%32]; ccss[p, 1, s] = sin[s, p%32]
    ct = np.tile(cos.T, (4, 1))  # [128, S]
    st_ = np.tile(sin.T, (4, 1))
    ccss = np.ascontiguousarray(np.stack([ct, st_], axis=1).astype(bf))
    sscc = np.ascontiguousarray(np.stack([st_, ct], axis=1).astype(bf))

    idm = np.eye(128, dtype=bf)
    kappa = np.arange(128)[:, None]
    u = np.arange(128)[None, :]
    maskc = np.where(u >= kappa, 0.0, MASK_VAL).astype(bf)

    xTs = [np.ascontiguousarray(hs[b].T.astype(bf)) for b in range(B)]

    in_maps = []
    for c in range(N_CORES):
        b, g = c // 4, c % 4
        hsl = slice(DG * g, DG * (g + 1))
        wq_g = Wq[:, hsl].reshape(D, HPC, DH)
        wk_g = Wk[:, hsl].reshape(D, HPC, DH)
        in_maps.append({
            "xT": xTs[b],
            "wq1": np.ascontiguousarray(
                wq_g[:, :, :32].reshape(D, 128).astype(bf)),
            "wq2": np.ascontiguousarray(
                wq_g[:, :, 32:].reshape(D, 128).astype(bf)),
            "wk1": np.ascontiguousarray(
                wk_g[:, :, :32].reshape(D, 128).astype(bf)),
            "wk2": np.ascontiguousarray(
                wk_g[:, :, 32:].reshape(D, 128).astype(bf)),
            "wv": np.ascontiguousarray(Wv[:, hsl].astype(bf)),
            "wo": np.ascontiguousarray(Wo[hsl, :].astype(bf)),
            "ccss": ccss,
            "sscc": sscc,
            "ident": idm,
            "maskc": maskc,
        })
    return in_maps


_NC_CACHE = {}


def get_nc():
    if "nc" not in _NC_CACHE:
        _NC_CACHE["nc"] = build_nc()
    return _NC_CACHE["nc"]


def run(inputs, trace=False):
    """Returns (output [B,S,D] fp32, BassKernelResults)."""
    nc = get_nc()
    in_maps = prep_inputs(**inputs)
    res = run_bass_kernel_spmd(nc, in_maps, list(range(N_CORES)), trace=trace)
    out = np.zeros((B, S, D), dtype=np.float32)
    for c in range(N_CORES):
        out[c // 4] += res.results[c]["o_part"].astype(np.float32)
    return out, res


def kernel(**inputs):
    return run(inputs, trace=False)[0]
